# revision 63
# baseline (speedup 1.0000x reference)
"""Trainium2 Bass kernel: causal self-attention with RoPE (B=2, T=2048, D=2048, H=16).

Sharding: 8 cores = 2-way data parallel over batch x 4-way tensor parallel over
heads.  Core c = 4*b + g computes batch b, heads 4g..4g+3, and produces a
partial output y_partial = attn_out[:, heads_g] @ w_proj[:, heads_g].T which the
host sums over g.

Per-core pipeline (v3 — fp8 DoubleRow for the projections):
  - qkv projection runs in fp8e4 with MatmulPerfMode.DoubleRow (2 K-planes per
    instruction at 0.5 cycles/row) using a host-side error-compensated split:
    w = wh + wl, x = xh + xl (each term an e4m3 tensor), accumulating
    wh*xh + wl*xh + wh*xl into the same psum (12N cycles per 128x2048-contract
    tile vs 16N in fp32r/bf16).  Weights are pre-scaled by 2^6 on the host so
    the hi/lo parts stay out of e4m3's subnormal floor; the 2^-6 is folded
    into the (host-precomputed) rope cos/sin for q/k, into the ones-columns of
    the V staging tiles (memset 64.0: den = 64*sum(p), o = num*(1/den)*64
    cancels), and into the final y drain (scaled copy) for the projection.
  - scores/PV stay bf16; score matmuls + exp narrow to the causal range on the
    diagonal j-tiles (the skipped top-left pt region is never read).
  - attention output oT is split to fp8 hi/lo on the (otherwise idle) GpSimd
    engine; the output projection is again 3-term DoubleRow over head pairs.
  - issue-level interleave: pair-1 qkv t-blocks are issued between h0/h1
    attention chunks so qkv matmuls fill the exp-latency PE gaps.
"""

import sys

import numpy as np
import ml_dtypes

for _p in ("/opt/trn_rl_repo", "/root/.axon_site/_ro/trn_rl_repo"):
    if _p not in sys.path:
        sys.path.append(_p)

import concourse.bass as bass
import concourse.bacc as bacc
import concourse.tile as tile
from concourse import mybir
from concourse.bass_utils import run_bass_kernel_spmd

F32 = mybir.dt.float32
BF16 = mybir.dt.bfloat16
FP8 = mybir.dt.float8e4
AF = mybir.ActivationFunctionType
DR = mybir.MatmulPerfMode.DoubleRow

B, T, D, H = 2, 2048, 2048, 16
HPC = H // 4  # heads per core (4-way head TP)
DH = D // H   # 128
SCALE = float(DH) ** -0.5
WS = 64.0     # host-side weight pre-scale (2^6)

TB = 512      # qkv-projection t-block (psum free width)
SB = 512      # attention i-block (score free width)


def build_nc(t=T, mm_dt="float32r", pv_dt="bfloat16"):
    """Build the SPMD per-core program.  `t` is the sequence length (smaller
    values are used for simulator validation).  mm_dt/pv_dt kept for test.py
    compatibility (ignored: matmuls are fp8-DoubleRow / bf16)."""
    NT = t // 128    # token tiles
    TBE = min(TB, t)
    NTB = t // TBE   # qkv t-blocks
    sb = min(SB, t)
    NSB = t // sb    # attention i-blocks
    NIC = sb // 128  # i-chunks per i-block
    ND = D // 128    # contraction d-tiles
    NJ = ND // 2     # DoubleRow K-pair count for the qkv contraction
    TTPB = TBE // 128

    nc = bacc.Bacc("TRN2", target_bir_lowering=False, debug=False)

    xTh = nc.dram_tensor("xTh", [D, t], FP8, kind="ExternalInput").ap()
    xTl = nc.dram_tensor("xTl", [D, t], FP8, kind="ExternalInput").ap()
    wqh_hi = nc.dram_tensor("wqh_hi", [128, HPC * D], FP8, kind="ExternalInput").ap()
    wqh_lo = nc.dram_tensor("wqh_lo", [128, HPC * D], FP8, kind="ExternalInput").ap()
    wkh_hi = nc.dram_tensor("wkh_hi", [128, HPC * D], FP8, kind="ExternalInput").ap()
    wkh_lo = nc.dram_tensor("wkh_lo", [128, HPC * D], FP8, kind="ExternalInput").ap()
    wvh_hi = nc.dram_tensor("wvh_hi", [128, (HPC // 2) * 2 * D], FP8, kind="ExternalInput").ap()
    wvh_lo = nc.dram_tensor("wvh_lo", [128, (HPC // 2) * 2 * D], FP8, kind="ExternalInput").ap()
    wpT_hi = nc.dram_tensor("wpT_hi", [HPC * DH, D], FP8, kind="ExternalInput").ap()
    wpT_lo = nc.dram_tensor("wpT_lo", [HPC * DH, D], FP8, kind="ExternalInput").ap()
    cosT = nc.dram_tensor("cosT", [DH, t], BF16, kind="ExternalInput").ap()
    sinTm = nc.dram_tensor("sinTm", [DH, t], BF16, kind="ExternalInput").ap()
    bmask = nc.dram_tensor("bmask", [128, 128], BF16, kind="ExternalInput").ap()
    ident = nc.dram_tensor("ident", [128, 128], BF16, kind="ExternalInput").ap()
    rotm = nc.dram_tensor("rotm", [128, 128], BF16, kind="ExternalInput").ap()
    y = nc.dram_tensor("y", [t, D], BF16, kind="ExternalOutput").ap()

    with tile.TileContext(nc) as tc:
        with (
            tc.tile_pool(name="consts", bufs=1) as cpool,
            tc.tile_pool(name="oTp", bufs=1) as opool,
            tc.tile_pool(name="qkp", bufs=2) as qkpool,
            tc.tile_pool(name="xtp", bufs=2) as xtp,
            tc.tile_pool(name="wqkp", bufs=1) as wqkp,
            tc.tile_pool(name="wvp", bufs=1) as wvp,
            tc.tile_pool(name="wpj", bufs=1) as wpj,
            tc.tile_pool(name="vep", bufs=2) as vep,
            tc.tile_pool(name="ptp", bufs=max(2 * NT + 4, NT + 1)) as ptp,
            tc.tile_pool(name="tmpp", bufs=6) as tmpp,
            tc.tile_pool(name="smallp", bufs=8) as smallp,
            tc.tile_pool(name="ysp", bufs=2) as ysp,
            tc.tile_pool(name="ps_mm", bufs=5, space="PSUM") as ps_mm,
            tc.tile_pool(name="ps_sm", bufs=2, space="PSUM") as ps_sm,
            tc.tile_pool(name="ps_rp", bufs=1, space="PSUM") as ps_rp,
        ):
            # PE warmup: dummy matmuls on a memset tile bridge the initial
            # DMA wait so the p-state ramp (half-rate for 3us after idle)
            # completes before the first real matmul.
            if t >= 2048:
                wrm = cpool.tile([128, 512], BF16, tag="wrm", name="wrm")
                nc.vector.memset(wrm[:], 0.0)
                for wi in range(9):
                    wps = ps_rp.tile([128, 512], F32, tag="rp", name=f"warm{wi}")
                    nc.tensor.matmul(wps[:], wrm[:, 0:128], wrm[:],
                                     start=True, stop=True)

            cos_sb = cpool.tile([DH, t], BF16, tag="cos")
            sin_sb = cpool.tile([DH, t], BF16, tag="sin")
            bm_sb = cpool.tile([128, 128], BF16, tag="bm")
            id_sb = cpool.tile([128, 128], BF16, tag="id")
            rot_sb = cpool.tile([128, 128], BF16, tag="rot")
            # fp8 hi/lo attention outputs, head-plane layout for DoubleRow
            oTh_sb = opool.tile([128, HPC, t], FP8, tag="oTh", name="oTh")
            oTl_sb = opool.tile([128, HPC, t], FP8, tag="oTl", name="oTl")
            wph_sb = wpj.tile([128, HPC, D], FP8, tag="wph", name="wph")
            wpl_sb = wpj.tile([128, HPC, D], FP8, tag="wpl", name="wpl")

            def rope_ip(qk, tb, name, eng=None):
                """In-place RoPE on qk[:, tb-block] (holds the WS-scaled raw
                projection).  cos/sin are host-scaled by 1/WS, so the result
                is the true-scale rotated q/k.  The half-rotation runs on the
                PE as a signed permutation matmul (own psum pool so it never
                steals a qkv/score bank).  `eng` picks the engine for the two
                elementwise muls: gpsimd for latency-tolerant in-loop ropes,
                vector for phase-boundary ones."""
                t0, t1_ = TBE * tb, TBE * (tb + 1)
                blk = qk[:, t0:t1_]
                # pre-sin form: rot(blk*sin) == rot(blk)*sin because RoPE's
                # emb duplicates the frequencies in both halves (sin[d] ==
                # sin[(d+64)%128]).  This leaves only one DVE op (the psum
                # add) after the PE rotation instead of two.
                if eng is None:
                    eng = nc.vector
                m2, r1 = rope_mul(qk, tb, name, eng)
                rope_fin(qk, tb, name, m2, r1)

            def rope_mul(qk, tb, name, eng=None):
                """The two elementwise rope muls — only depend on the parked
                q/k block, so they can issue right after the park while the
                rot+add defer to the next schedule slot."""
                t0, t1_ = TBE * tb, TBE * (tb + 1)
                blk = qk[:, t0:t1_]
                if eng is None:
                    eng = nc.vector
                m2 = tmpp.tile([128, TBE], BF16, tag="r2", name=f"m2_{name}")
                eng.tensor_mul(m2[:], blk, sin_sb[:, t0:t1_])
                r1 = tmpp.tile([128, TBE], BF16, tag="r1", name=f"r1_{name}")
                eng.tensor_mul(r1[:], blk, cos_sb[:, t0:t1_])
                return m2, r1

            def rope_fin(qk, tb, name, m2, r1):
                t0, t1_ = TBE * tb, TBE * (tb + 1)
                blk = qk[:, t0:t1_]
                rps = ps_rp.tile([128, TBE], F32, tag="rp", name=f"rot_{name}")
                nc.tensor.matmul(rps[:], rot_sb[:], m2[:], start=True, stop=True)
                nc.vector.tensor_add(blk, r1[:], rps[:])

            def emit_proj(tt, on_act=False, tail=False):
                """y[128*tt:128*(tt+1), :] = sum_h oT_h[:, tt].T @ wp_h, as
                3-term fp8 DoubleRow over head pairs.  Psum carries WS*y;
                the drains (split across Act/DVE) fold in 1/WS.  One DMA per
                row-block normally; per-512-chunk DMAs on the tail so the
                last transfer isn't a serial 1MB copy."""
                yst = ysp.tile([128, D], BF16, tag="yst", name=f"yst{tt}")
                for db in range(D // 512):
                    ps = ps_mm.tile([128, 512], F32, tag="mm", name=f"psy{tt}_{db}")
                    idx = 0
                    for lhs_t, rhs_t in ((oTh_sb, wph_sb), (oTl_sb, wph_sb), (oTh_sb, wpl_sb)):
                        for g in range(HPC // 2):
                            nc.tensor.matmul(
                                ps[:],
                                lhs_t[:, 2 * g:2 * g + 2, 128 * tt:128 * (tt + 1)],
                                rhs_t[:, 2 * g:2 * g + 2, 512 * db:512 * (db + 1)],
                                start=(idx == 0), stop=(idx == 3 * (HPC // 2) - 1),
                                perf_mode=DR)
                            idx += 1
                    ys = yst[:, 512 * db:512 * (db + 1)]
                    if tail and db % 2 == (0 if on_act else 1):
                        nc.scalar.mul(ys, ps[:], 1.0 / WS)
                    else:
                        # keep the Act queue free for exps: non-tail drains
                        # all run on DVE
                        nc.vector.tensor_scalar_mul(ys, ps[:], 1.0 / WS)
                    if tail:
                        nc.sync.dma_start(
                            y[128 * tt:128 * (tt + 1), 512 * db:512 * (db + 1)], ys)
                if not tail:
                    nc.sync.dma_start(y[128 * tt:128 * (tt + 1), :], yst[:])

            pair_bufs = {}
            pend_ropes = []

            def qkv_pair_steps(p2):
                """Generator: pair-p2 weight/x DMAs + fp8 qkv projection;
                yields once per t-block."""
                h = 2 * p2
                q_sbs = [None, None]
                k_sbs = [None, None]
                vext = [None, None]
                # fp8 weight tiles, [128, ND, 128]-shaped for DoubleRow pairs
                wq = [[wqkp.tile([128, ND, 128], FP8, tag=f"wq{i}{lv}", name=f"wq{i}{lv}_{p2}")
                       for lv in range(2)] for i in range(2)]
                wk = [[wqkp.tile([128, ND, 128], FP8, tag=f"wk{i}{lv}", name=f"wk{i}{lv}_{p2}")
                       for lv in range(2)] for i in range(2)]
                wv = [wvp.tile([128, ND, 256], FP8, tag=f"wv{lv}", name=f"wv{lv}_{p2}")
                      for lv in range(2)]
                vext[0] = vep.tile([128, NT, 129], BF16, tag="ve0", name=f"ve0_{p2}")
                vext[1] = vep.tile([128, NT, 129], BF16, tag="ve1", name=f"ve1_{p2}")
                # only the ones-columns need init (v-parks overwrite the rest);
                # 64.0 folds the WS weight scale out of the normalizer.
                nc.vector.memset(vext[0][:, :, 128:129], WS)
                nc.vector.memset(vext[1][:, :, 128:129], WS)
                q_sbs[0] = qkpool.tile([DH, t], BF16, tag="q0", name=f"q0_{p2}")
                k_sbs[0] = qkpool.tile([DH, t], BF16, tag="k0", name=f"k0_{p2}")
                q_sbs[1] = qkpool.tile([DH, t], BF16, tag="q1", name=f"q1_{p2}")
                k_sbs[1] = qkpool.tile([DH, t], BF16, tag="k1", name=f"k1_{p2}")
                pair_bufs[p2] = {"q": q_sbs, "k": k_sbs, "ve": vext}

                # x tiles for this pair (double-buffered across t-blocks)
                xts = {}
                loop_ropes = []

                def dma_x(tb):
                    xh_t = xtp.tile([128, ND, TBE], FP8, tag="xh", name=f"xh{p2}_{tb}")
                    xl_t = xtp.tile([128, ND, TBE], FP8, tag="xl", name=f"xl{p2}_{tb}")
                    t0, t1_ = TBE * tb, TBE * (tb + 1)
                    hd = ND // 2
                    for hf in range(2):
                        nc.sync.dma_start(
                            xh_t[:, hd * hf:hd * (hf + 1), :],
                            xTh[1024 * hf:1024 * (hf + 1), t0:t1_].rearrange(
                                "(j p) c -> p j c", p=128))
                    for hf in range(2):
                        nc.sync.dma_start(
                            xl_t[:, hd * hf:hd * (hf + 1), :],
                            xTl[1024 * hf:1024 * (hf + 1), t0:t1_].rearrange(
                                "(j p) c -> p j c", p=128))
                    xts[tb] = (xh_t, xl_t)

                # DMA order tuned so the first q0 matmul starts ~3us in and
                # every later consumer arrives just ahead of its first use:
                # wq0h, x-hi, wk0h, wq1h/wk1h, x-lo, lo-weights, wv.
                w0 = D * h
                w1 = D * (h + 1)
                xh_t = xtp.tile([128, ND, TBE], FP8, tag="xh", name=f"xh{p2}_0")
                xl_t = xtp.tile([128, ND, TBE], FP8, tag="xl", name=f"xl{p2}_0")
                hd = ND // 2
                qd = ND // 4
                nc.sync.dma_start(wq[0][0][:], wqh_hi[:, w0:w1])
                nc.sync.dma_start(
                    xh_t[:, 0:qd, :],
                    xTh[0:512, 0:TBE].rearrange("(j p) c -> p j c", p=128))
                nc.sync.dma_start(
                    xh_t[:, qd:2 * qd, :],
                    xTh[512:1024, 0:TBE].rearrange("(j p) c -> p j c", p=128))
                nc.sync.dma_start(wk[0][0][:], wkh_hi[:, w0:w1])
                nc.sync.dma_start(
                    xh_t[:, 2 * qd:3 * qd, :],
                    xTh[1024:1536, 0:TBE].rearrange("(j p) c -> p j c", p=128))
                nc.sync.dma_start(
                    xh_t[:, 3 * qd:ND, :],
                    xTh[1536:2048, 0:TBE].rearrange("(j p) c -> p j c", p=128))
                nc.sync.dma_start(wq[1][0][:], wqh_hi[:, w1:w1 + D])
                nc.sync.dma_start(wk[1][0][:], wkh_hi[:, w1:w1 + D])
                # term-1 lo weights arrive before term-2's x-lo
                nc.sync.dma_start(wq[0][1][:], wqh_lo[:, w0:w1])
                nc.sync.dma_start(wk[0][1][:], wkh_lo[:, w0:w1])
                nc.sync.dma_start(wq[1][1][:], wqh_lo[:, w1:w1 + D])
                nc.sync.dma_start(wk[1][1][:], wkh_lo[:, w1:w1 + D])
                for hf in range(2):
                    nc.sync.dma_start(
                        xl_t[:, hd * hf:hd * (hf + 1), :],
                        xTl[1024 * hf:1024 * (hf + 1), 0:TBE].rearrange(
                            "(j p) c -> p j c", p=128))
                xts[0] = (xh_t, xl_t)
                if p2 == 0:
                    # only tb0's cos/sin chunk up front; later chunks stream
                    # per-t-block so the x prefetches aren't queued behind
                    # 1MB of consts on the serial DMA device
                    nc.sync.dma_start(cos_sb[:, 0:TBE], cosT[:, 0:TBE])
                    nc.sync.dma_start(sin_sb[:, 0:TBE], sinTm[:, 0:TBE])
                nc.sync.dma_start(wv[0][:], wvh_hi[:, 2 * D * p2:2 * D * (p2 + 1)])
                nc.sync.dma_start(wv[1][:], wvh_lo[:, 2 * D * p2:2 * D * (p2 + 1)])
                if p2 == 0:
                    nc.sync.dma_start(rot_sb[:], rotm[:])
                    nc.sync.dma_start(bm_sb[:], bmask[:])
                    nc.sync.dma_start(id_sb[:], ident[:])
                else:
                    # prefetch the projection weights during pair-1 qkv
                    for hh in range(HPC):
                        nc.sync.dma_start(
                            wph_sb[:, hh, :], wpT_hi[128 * hh:128 * (hh + 1), :])
                        nc.sync.dma_start(
                            wpl_sb[:, hh, :], wpT_lo[128 * hh:128 * (hh + 1), :])

                for tb in range(NTB):
                    t0, t1_ = TBE * tb, TBE * (tb + 1)
                    for qk, tb_, nm_, m2_, r1_ in loop_ropes:
                        rope_fin(qk, tb_, nm_, m2_, r1_)
                    loop_ropes.clear()
                    if tb + 1 < NTB:
                        dma_x(tb + 1)
                        if p2 == 0:
                            nt0, nt1 = TBE * (tb + 1), TBE * (tb + 2)
                            nc.sync.dma_start(cos_sb[:, nt0:nt1], cosT[:, nt0:nt1])
                            nc.sync.dma_start(sin_sb[:, nt0:nt1], sinTm[:, nt0:nt1])
                    xh_t, xl_t = xts.pop(tb)
                    ps_q0 = ps_mm.tile([128, TBE], F32, tag="mm", name=f"psq0_{p2}_{tb}")
                    ps_k0 = ps_mm.tile([128, TBE], F32, tag="mm", name=f"psk0_{p2}_{tb}")
                    ps_q1 = ps_mm.tile([128, TBE], F32, tag="mm", name=f"psq1_{p2}_{tb}")
                    ps_k1 = ps_mm.tile([128, TBE], F32, tag="mm", name=f"psk1_{p2}_{tb}")
                    nvp = (TTPB + 1) // 2
                    ps_vs = [
                        ps_sm.tile([128, 512], F32, tag="sm", name=f"psv{p2}_{tb}_{i}")
                        for i in range(nvp)
                    ]
                    # 3-term fp8 accumulation: wh*xh + wl*xh + wh*xl,
                    # term-major (matches the tb-0 DMA arrival order); each
                    # psum parks right after its last matmul so the bank
                    # frees and the copy overlaps the remaining matmuls.
                    last_tb = tb == NTB - 1
                    qk_psums = ((ps_q0, wq[0], q_sbs[0]), (ps_k0, wk[0], k_sbs[0]),
                                (ps_q1, wq[1], q_sbs[1]), (ps_k1, wk[1], k_sbs[1]))

                    def qk_sweep():
                        for term in range(3):
                            wlv = 1 if term == 1 else 0
                            xt = xl_t if term == 2 else xh_t
                            first = term == 0
                            last = term == 2
                            for i_, (ps, wt, dst) in enumerate(qk_psums):
                                for j in range(NJ):
                                    js = slice(2 * j, 2 * j + 2)
                                    nc.tensor.matmul(ps[:], wt[wlv][:, js, :], xt[:, js, :],
                                                     start=(first and j == 0),
                                                     stop=(last and j == NJ - 1),
                                                     perf_mode=DR)
                                if last:
                                    # split parks across Act/DVE once
                                    # attention exps share the Act queue
                                    if tb > 0 and i_ % 2 == 1:
                                        nc.vector.tensor_copy(dst[:, t0:t1_], ps[:])
                                    else:
                                        nc.scalar.copy(dst[:, t0:t1_], ps[:])

                    def v_sweep():
                        for term in range(3):
                            wvt = wv[1 if term == 1 else 0]
                            xt = xl_t if term == 2 else xh_t
                            first = term == 0
                            last = term == 2
                            for j in range(NJ):
                                js = slice(2 * j, 2 * j + 2)
                                for tt in range(TTPB):
                                    nc.tensor.matmul(
                                        ps_vs[tt // 2][:, 256 * (tt % 2):256 * (tt % 2) + 256],
                                        xt[:, js, 128 * tt:128 * (tt + 1)],
                                        wvt[:, js, :],
                                        start=(first and j == 0 and tt % 2 == 0),
                                        stop=(last and j == NJ - 1),
                                        skip_group_check=True, perf_mode=DR)
                        for tt in range(TTPB):
                            gt = tb * TTPB + tt
                            o0 = 256 * (tt % 2)
                            vc = nc.vector.tensor_copy if (tb > 0 and tt % 2) else nc.scalar.copy
                            vc(vext[0][:, gt, 0:128], ps_vs[tt // 2][:, o0:o0 + 128])
                            vc(vext[1][:, gt, 0:128], ps_vs[tt // 2][:, o0 + 128:o0 + 256])

                    # last t-block: V first so its psums drain under the q/k
                    # matmuls and the pair boundary ends on parks+ropes only
                    if last_tb and tb > 0:
                        v_sweep()
                        qk_sweep()
                    else:
                        qk_sweep()
                        v_sweep()
                    # RoPE policy: in-loop ropes are queued and ISSUED at the
                    # start of the next t-block slot, so their rot matmuls
                    # land ahead of the 15us qkv stream in the in-order PE
                    # queue (a rope add stuck behind a whole t-block blocks
                    # every later DVE op).  Pair 0 rotates all four heads
                    # in-loop; pair 1 only its h2 head (h3's rotate in phase
                    # 3 via scheduler-placed consume_ropes).  The last
                    # t-block always defers.
                    if tb != NTB - 1:
                        heads = [0, 1] if p2 == 1 else [0]
                        for par in heads:
                            for qk, pfx in ((q_sbs[par], "q"), (k_sbs[par], "k")):
                                nm = f"{pfx}{par}_{p2}_{tb}"
                                m2_, r1_ = rope_mul(qk, tb, nm, eng=nc.gpsimd)
                                loop_ropes.append((qk, tb, nm, m2_, r1_))
                        if p2 == 0:
                            pend_ropes.append((q_sbs[1], tb, f"q1_{p2}_{tb}"))
                            pend_ropes.append((k_sbs[1], tb, f"k1_{p2}_{tb}"))
                    else:
                        # the pair's own (h-even) tail ropes go to the FRONT
                        # so schedule-placed consume_ropes() can rotate them
                        # before the partner-head ones
                        pend_ropes.insert(0, (k_sbs[0], tb, f"k0_{p2}_{tb}"))
                        pend_ropes.insert(0, (q_sbs[0], tb, f"q0_{p2}_{tb}"))
                        pend_ropes.append((q_sbs[1], tb, f"q1_{p2}_{tb}"))
                        pend_ropes.append((k_sbs[1], tb, f"k1_{p2}_{tb}"))
                    yield

            def attention_steps(h, quotas, pipelined, do_proj=False):
                """Generator for head h's attention, yielding once per chunk."""
                par = h % 2
                bufs = pair_bufs[h // 2]
                q_sb, k_sb = bufs["q"][par], bufs["k"][par]
                ve = bufs["ve"][par]
                if par == 0 and pend_ropes:
                    mine = [e for e in pend_ropes if e[0] is q_sb or e[0] is k_sb]
                    rest = [e for e in pend_ropes if not (e[0] is q_sb or e[0] is k_sb)]
                    pend_ropes[:] = mine + rest

                def stage_a(ib):
                    """scores + exp + diagonal mask for i-block ib; the score
                    matmul and exp narrow to the causal range on diagonal
                    j-tiles."""
                    i0 = sb * ib
                    jt_max = (i0 + sb) // 128 - 1  # inclusive
                    pts = [None] * (jt_max + 1)
                    for jt in range(jt_max + 1):
                        m = jt - NIC * ib
                        off = 128 * m if m > 0 else 0
                        s_ps = ps_mm.tile([128, sb], F32, tag="mm", name=f"s{h}_{ib}_{jt}")
                        nc.tensor.matmul(
                            s_ps[:, off:sb],
                            k_sb[:, 128 * jt:128 * (jt + 1)],
                            q_sb[:, i0 + off:i0 + sb],
                            start=True, stop=True)
                        pt_t = ptp.tile([128, sb], BF16, tag="pt", name=f"pt{h}_{ib}_{jt}")
                        nc.scalar.activation(pt_t[:, off:sb], s_ps[:, off:sb], AF.Exp, scale=SCALE)
                        if m >= 0:
                            pm = pt_t[:, 128 * m:128 * (m + 1)]
                            nc.vector.tensor_mul(pm, pm, bm_sb[:])
                        pts[jt] = pt_t
                    return pts

                def stage_b(ib, pts, bi):
                    """PV + normalize + transpose + fp8 hi/lo split."""
                    i0 = sb * ib

                    def finish(ic, pv):
                        rc = smallp.tile([128, 1], F32, tag="rc", name=f"rc{h}_{ib}_{ic}")
                        nc.vector.reciprocal(rc[:], pv[:, 128:129])
                        o_sb = smallp.tile([128, 128], BF16, tag="o", name=f"o{h}_{ib}_{ic}")
                        nc.vector.tensor_scalar_mul(o_sb[:], pv[:, 0:128], rc[:])
                        ot_ps = ps_mm.tile([128, 128], BF16, tag="mm", name=f"otp{h}_{ib}_{ic}")
                        nc.tensor.transpose(ot_ps[:], o_sb[:], id_sb[:])
                        c0 = i0 + 128 * ic
                        hs = oTh_sb[:, h, c0:c0 + 128]
                        if do_proj:
                            # proj-critical head: split straight off the psum
                            # on Act + DVE (no Pool latency in the chain)
                            nc.scalar.copy(hs, ot_ps[:])
                            nc.vector.scalar_tensor_tensor(
                                oTl_sb[:, h, c0:c0 + 128], hs, -1.0, ot_ps[:],
                                mybir.AluOpType.mult, mybir.AluOpType.add)
                        else:
                            otb = smallp.tile([128, 128], BF16, tag="otb", name=f"otb{h}_{ib}_{ic}")
                            nc.vector.tensor_copy(otb[:], ot_ps[:])
                            # fp8 hi/lo split on GpSimd (SBUF-only engine)
                            nc.gpsimd.tensor_copy(hs, otb[:])
                            nc.gpsimd.tensor_sub(oTl_sb[:, h, c0:c0 + 128], otb[:], hs)

                    prev = None
                    for ic in range(NIC):
                        last_jt = NIC * ib + ic
                        pv = ps_sm.tile([128, 129], F32, tag="sm", name=f"pv{h}_{ib}_{ic}")
                        for jt in range(last_jt + 1):
                            nc.tensor.matmul(
                                pv[:],
                                pts[jt][:, 128 * ic:128 * (ic + 1)],
                                ve[:, jt, :],
                                start=(jt == 0), stop=(jt == last_jt))
                        if prev is not None:
                            finish(*prev)
                            if do_proj:
                                emit_proj(NIC * ib + prev[0], on_act=False,
                                          tail=(ib == NSB - 1))
                        prev = (ic, pv)
                    finish(*prev)
                    if do_proj:
                        emit_proj(NIC * ib + prev[0],
                                  on_act=(ib == NSB - 1), tail=(ib == NSB - 1))
                    consume_ropes(quotas[bi])

                if pipelined:
                    pts_prev = None
                    for ib in range(NSB):
                        pts_cur = stage_a(ib)
                        yield
                        if pts_prev is not None:
                            stage_b(ib - 1, pts_prev, ib - 1)
                            yield
                        pts_prev = pts_cur
                    stage_b(NSB - 1, pts_prev, NSB - 1)
                    yield
                else:
                    for ib in range(NSB):
                        pts = stage_a(ib)
                        yield
                        stage_b(ib, pts, ib)
                        yield

            def stepn(g, n):
                for _ in range(n):
                    next(g)

            def run(g):
                for _ in g:
                    pass

            def consume_ropes(n):
                for qk, tb_, nm_ in pend_ropes[:n]:
                    rope_ip(qk, tb_, nm_, eng=nc.gpsimd)
                del pend_ropes[:n]

            # ---- schedule -------------------------------------------
            # pair 0 qkv alone; h0/h1 attention chunks (mutually interleaved
            # so one head's scores fill the other's exp latency) interleaved
            # with pair 1's qkv t-blocks; h2/h3 likewise interleaved with
            # each other and the output projection folded into h3's B chunks.
            if NTB >= 4 and NSB >= 4:
                Z = [0, 0, 0, 0]
                # h0's attention starts one i-block behind pair-0's qkv
                # t-blocks (its k/v prefix is complete by then), filling
                # pair-0's otherwise idle Act with exps; deferred ropes are
                # consumed at explicit schedule points, each before any chunk
                # that reads the rotated tile.
                q0 = qkv_pair_steps(0)
                a0 = attention_steps(0, Z, True)
                stepn(q0, 2)        # tb0 tb1 (tb0's ropes issue at tb1 start)
                stepn(a0, 1)        # A0
                stepn(q0, 1)        # tb2 (ropes tb1)
                stepn(a0, 2)        # A1 B0
                stepn(q0, 1)        # tb3 (ropes tb2)
                consume_ropes(2)    # q0/k0 pair0-tb3
                stepn(a0, 3)        # A2 B1 A3
                consume_ropes(4)    # q1/k1 pair0-tb0, tb1
                a1 = attention_steps(1, Z, True)
                q1 = qkv_pair_steps(1)
                a2 = attention_steps(2, Z, True)
                stepn(a0, 1)        # B2
                stepn(a1, 1)        # A0
                stepn(q1, 1)        # pair1 tb0
                consume_ropes(2)    # q1/k1 pair0-tb2
                stepn(a1, 2)        # A1 B0
                stepn(a0, 1)        # B3
                stepn(q1, 1)        # tb1 (ropes p1-tb0)
                consume_ropes(2)    # q1/k1 pair0-tb3
                stepn(a2, 1)        # h2.A0
                stepn(a1, 2)        # A2 B1
                stepn(q1, 1)        # tb2 (ropes p1-tb1)
                stepn(a2, 2)        # h2.A1 B0
                stepn(a1, 2)        # A3 B2
                stepn(q1, 1)        # tb3 (ropes p1-tb2)
                consume_ropes(2)    # q0/k0 pair1-tb3
                stepn(a2, 2)        # h2.A2 B1
                run(a1)             # B3
                run(a0)
                run(q0)
                run(q1)
                # h3's first chunks (and ib0's projection) pull into the
                # phase-2 tail where Act still has headroom; phase 3 is the
                # remainder with h2's tail interleaved.
                a3 = attention_steps(3, Z, True, do_proj=True)
                consume_ropes(2)    # q1/k1 pair1-tb3
                stepn(a3, 3)        # A0 A1 B0 (+proj ib0)
                stepn(a2, 1)        # h2.A3
                stepn(a2, 1)        # h2.B2
                stepn(a3, 2)        # A2 B1 (+proj ib1)
                stepn(a2, 1)        # h2.B3
                stepn(a3, 1)        # A3
                run(a3)             # B2 B3 (+proj ib2, ib3)
                run(a2)
            else:
                run(qkv_pair_steps(0))
                run(attention_steps(0, [2, 2, 0, 0], True))
                run(attention_steps(1, [0, 0, 0, 0], True))
                run(qkv_pair_steps(1))
                run(attention_steps(2, [2, 2, 0, 0], True))
                run(attention_steps(3, [0, 0, 0, 0], True, do_proj=True))

    nc.compile()
    return nc


def host_consts(t=T):
    """RoPE cos/sin (scaled by 1/WS to fold out the fp8 weight pre-scale),
    causal big-mask, identity, signed half-rotation."""
    inv = (1.0 / (np.float32(10000.0) ** (np.arange(0, DH, 2, dtype=np.float32) / np.float32(DH)))).astype(np.float32)
    tt = np.arange(t, dtype=np.float32)
    fr = np.outer(tt, inv).astype(np.float32)       # [t, 64]
    emb = np.concatenate([fr, fr], axis=1)          # [t, 128]
    cosT = np.ascontiguousarray(np.cos(emb).T.astype(np.float32)) / np.float32(WS)
    sinTm = np.ascontiguousarray(np.sin(emb).T.astype(np.float32)) / np.float32(WS)
    jj = np.arange(128)[:, None]
    cc = np.arange(128)[None, :]
    bmask = (cc >= jj).astype(np.float32)
    ident = np.eye(128, dtype=np.float32)
    # signed half-rotation: (rotm.T @ x)[d] = -x[d+64] for d<64, x[d-64] else
    rotm = np.zeros((128, 128), dtype=np.float32)
    for d in range(64):
        rotm[d + 64, d] = -1.0
        rotm[d, d + 64] = 1.0
    return cosT, sinTm, bmask, ident, rotm


def _warrange(w):
    """[128*nh rows, D] head-major weight slice -> [128, nh*D] sbuf-ready layout:
    block h, col di*128+c of partition p  =  w[128*h + c, 128*di + p]."""
    nh = w.shape[0] // 128
    d = w.shape[1]
    out = np.empty((128, nh * d), dtype=w.dtype)
    for h in range(nh):
        a = w[128 * h:128 * (h + 1), :].T.reshape(d // 128, 128, 128)  # [di, p, c]
        out[:, d * h:d * (h + 1)] = a.transpose(1, 0, 2).reshape(128, d)
    return out


def _wvarrange(w):
    """[512 rows, D] 4-head v-weights -> [128, 2*2*D]: per pair, di-major blocks of
    [even-head 128 cols | odd-head 128 cols]."""
    d = w.shape[1]
    blocks = []
    for p2 in range(2):
        e = w[256 * p2:256 * p2 + 128, :].T.reshape(d // 128, 128, 128)
        o = w[256 * p2 + 128:256 * p2 + 256, :].T.reshape(d // 128, 128, 128)
        pair = np.concatenate([e, o], axis=2)          # [di, p, 256]
        blocks.append(pair.transpose(1, 0, 2).reshape(128, 2 * d))
    return np.concatenate(blocks, axis=1)


FP8NP = ml_dtypes.float8_e4m3


def _split8(a):
    """Error-compensated e4m3 split: a ~= hi + lo (fp32 in, fp8 out pair)."""
    a = np.asarray(a, dtype=np.float32)
    hi = a.astype(FP8NP)
    lo = (a - hi.astype(np.float32)).astype(FP8NP)
    return hi, lo


def shard_inputs(x, w_qkv, w_proj, t=T, pv_dt="bfloat16"):
    """Build the 8 per-core input maps (host does the fp8 splits)."""
    bdt = ml_dtypes.bfloat16
    cosT, sinTm, bmask, ident, rotm = host_consts(t)
    cosT = cosT.astype(bdt)
    sinTm = sinTm.astype(bdt)
    bmask = bmask.astype(bdt)
    ident = ident.astype(bdt)
    rotm = rotm.astype(bdt)
    d = x.shape[2]
    ws = np.float32(WS)
    in_maps = []
    xs = {}
    for b in range(x.shape[0]):
        xs[b] = _split8(np.ascontiguousarray(x[b].T))
    for c in range(8):
        b, g = divmod(c, 4)
        s0, s1 = 512 * g, 512 * (g + 1)
        wq_hi, wq_lo = _split8(w_qkv[s0:s1, :] * ws)
        wk_hi, wk_lo = _split8(w_qkv[d + s0:d + s1, :] * ws)
        wv_hi, wv_lo = _split8(w_qkv[2 * d + s0:2 * d + s1, :] * ws)
        wp_hi, wp_lo = _split8(np.ascontiguousarray(w_proj[:, s0:s1].T) * ws)
        in_maps.append(dict(
            xTh=xs[b][0], xTl=xs[b][1],
            wqh_hi=_warrange(wq_hi), wqh_lo=_warrange(wq_lo),
            wkh_hi=_warrange(wk_hi), wkh_lo=_warrange(wk_lo),
            wvh_hi=_wvarrange(wv_hi), wvh_lo=_wvarrange(wv_lo),
            wpT_hi=wp_hi, wpT_lo=wp_lo,
            cosT=cosT, sinTm=sinTm, bmask=bmask, ident=ident, rotm=rotm,
        ))
    return in_maps


_NC_CACHE = {}


def get_nc(t=T, mm_dt="float32r", pv_dt="bfloat16"):
    key = (t, mm_dt, pv_dt)
    if key not in _NC_CACHE:
        _NC_CACHE[key] = build_nc(t=t, mm_dt=mm_dt, pv_dt=pv_dt)
    return _NC_CACHE[key]


def kernel(x, w_qkv, w_proj):
    x = np.asarray(x, dtype=np.float32)
    w_qkv = np.asarray(w_qkv, dtype=np.float32)
    w_proj = np.asarray(w_proj, dtype=np.float32)
    b_, t_, d_ = x.shape
    in_maps = shard_inputs(x, w_qkv, w_proj, t=t_)
    nc = get_nc(t=t_)
    res = run_bass_kernel_spmd(nc, in_maps, list(range(8))).results
    out = np.zeros((b_, t_, d_), dtype=np.float32)
    for c in range(8):
        b, _ = divmod(c, 4)
        out[b] += res[c]["y"]
    return out


# revision 65
# speedup vs baseline: 1.0695x; 1.0695x over previous
"""Trainium2 Bass kernel: causal self-attention with RoPE (B=2, T=2048, D=2048, H=16).

Sharding: 8 cores = 2-way data parallel over batch x 4-way tensor parallel over
heads.  Core c = 4*b + g computes batch b, heads 4g..4g+3, and produces a
partial output y_partial = attn_out[:, heads_g] @ w_proj[:, heads_g].T which the
host sums over g.

Per-core pipeline (v3 — fp8 DoubleRow for the projections):
  - qkv projection runs in fp8e4 with MatmulPerfMode.DoubleRow (2 K-planes per
    instruction at 0.5 cycles/row) using a host-side error-compensated split:
    w = wh + wl, x = xh + xl (each term an e4m3 tensor), accumulating
    wh*xh + wl*xh + wh*xl into the same psum (12N cycles per 128x2048-contract
    tile vs 16N in fp32r/bf16).  Weights are pre-scaled by 2^6 on the host so
    the hi/lo parts stay out of e4m3's subnormal floor; the 2^-6 is folded
    into the (host-precomputed) rope cos/sin for q/k, into the ones-columns of
    the V staging tiles (memset 64.0: den = 64*sum(p), o = num*(1/den)*64
    cancels), and into the final y drain (scaled copy) for the projection.
  - scores/PV stay bf16; score matmuls + exp narrow to the causal range on the
    diagonal j-tiles (the skipped top-left pt region is never read).
  - attention output oT is split to fp8 hi/lo on the (otherwise idle) GpSimd
    engine; the output projection is again 3-term DoubleRow over head pairs.
  - issue-level interleave: pair-1 qkv t-blocks are issued between h0/h1
    attention chunks so qkv matmuls fill the exp-latency PE gaps.
"""

import sys

import numpy as np
import ml_dtypes

for _p in ("/opt/trn_rl_repo", "/root/.axon_site/_ro/trn_rl_repo"):
    if _p not in sys.path:
        sys.path.append(_p)

import concourse.bass as bass
import concourse.bacc as bacc
import concourse.tile as tile
from concourse import mybir
from concourse.bass_utils import run_bass_kernel_spmd

F32 = mybir.dt.float32
BF16 = mybir.dt.bfloat16
FP8 = mybir.dt.float8e4
AF = mybir.ActivationFunctionType
DR = mybir.MatmulPerfMode.DoubleRow

B, T, D, H = 2, 2048, 2048, 16
HPC = H // 4  # heads per core (4-way head TP)
DH = D // H   # 128
SCALE = float(DH) ** -0.5
WS = 64.0     # host-side weight pre-scale (2^6)

TB = 512      # qkv-projection t-block (psum free width)
SB = 512      # attention i-block (score free width)


def build_nc(t=T, mm_dt="float32r", pv_dt="bfloat16"):
    """Build the SPMD per-core program.  `t` is the sequence length (smaller
    values are used for simulator validation).  mm_dt/pv_dt kept for test.py
    compatibility (ignored: matmuls are fp8-DoubleRow / bf16)."""
    NT = t // 128    # token tiles
    TBE = min(TB, t)
    NTB = t // TBE   # qkv t-blocks
    sb = min(SB, t)
    NSB = t // sb    # attention i-blocks
    NIC = sb // 128  # i-chunks per i-block
    ND = D // 128    # contraction d-tiles
    NJ = ND // 2     # DoubleRow K-pair count for the qkv contraction
    TTPB = TBE // 128

    nc = bacc.Bacc("TRN2", target_bir_lowering=False, debug=False)

    xTh = nc.dram_tensor("xTh", [D, t], FP8, kind="ExternalInput").ap()
    xTl = nc.dram_tensor("xTl", [D, t], FP8, kind="ExternalInput").ap()
    wqh_hi = nc.dram_tensor("wqh_hi", [128, HPC * D], FP8, kind="ExternalInput").ap()
    wqh_lo = nc.dram_tensor("wqh_lo", [128, HPC * D], FP8, kind="ExternalInput").ap()
    wkh_hi = nc.dram_tensor("wkh_hi", [128, HPC * D], FP8, kind="ExternalInput").ap()
    wkh_lo = nc.dram_tensor("wkh_lo", [128, HPC * D], FP8, kind="ExternalInput").ap()
    wvh_hi = nc.dram_tensor("wvh_hi", [128, (HPC // 2) * 2 * D], FP8, kind="ExternalInput").ap()
    wvh_lo = nc.dram_tensor("wvh_lo", [128, (HPC // 2) * 2 * D], FP8, kind="ExternalInput").ap()
    wpT_hi = nc.dram_tensor("wpT_hi", [HPC * DH, D], FP8, kind="ExternalInput").ap()
    wpT_lo = nc.dram_tensor("wpT_lo", [HPC * DH, D], FP8, kind="ExternalInput").ap()
    cosT = nc.dram_tensor("cosT", [DH, t], BF16, kind="ExternalInput").ap()
    sinTm = nc.dram_tensor("sinTm", [DH, t], BF16, kind="ExternalInput").ap()
    bmask = nc.dram_tensor("bmask", [128, 128], BF16, kind="ExternalInput").ap()
    ident = nc.dram_tensor("ident", [128, 128], BF16, kind="ExternalInput").ap()
    rotm = nc.dram_tensor("rotm", [128, 128], BF16, kind="ExternalInput").ap()
    y = nc.dram_tensor("y", [t, D], BF16, kind="ExternalOutput").ap()

    with tile.TileContext(nc) as tc:
        with (
            tc.tile_pool(name="consts", bufs=1) as cpool,
            tc.tile_pool(name="oTp", bufs=1) as opool,
            tc.tile_pool(name="qkp", bufs=2) as qkpool,
            tc.tile_pool(name="xtp", bufs=2) as xtp,
            tc.tile_pool(name="wqkp", bufs=1) as wqkp,
            tc.tile_pool(name="wvp", bufs=1) as wvp,
            tc.tile_pool(name="wpj", bufs=1) as wpj,
            tc.tile_pool(name="vep", bufs=2) as vep,
            tc.tile_pool(name="ptp", bufs=max(2 * NT + 4, NT + 1)) as ptp,
            tc.tile_pool(name="tmpp", bufs=6) as tmpp,
            tc.tile_pool(name="smallp", bufs=8) as smallp,
            tc.tile_pool(name="ysp", bufs=2) as ysp,
            tc.tile_pool(name="ps_mm", bufs=5, space="PSUM") as ps_mm,
            tc.tile_pool(name="ps_sm", bufs=2, space="PSUM") as ps_sm,
            tc.tile_pool(name="ps_rp", bufs=1, space="PSUM") as ps_rp,
        ):
            # PE warmup: dummy matmuls on a memset tile bridge the initial
            # DMA wait so the p-state ramp (half-rate for 3us after idle)
            # completes before the first real matmul.
            if t >= 2048:
                wrm = cpool.tile([128, 512], BF16, tag="wrm", name="wrm")
                nc.vector.memset(wrm[:], 0.0)
                for wi in range(9):
                    wps = ps_rp.tile([128, 512], F32, tag="rp", name=f"warm{wi}")
                    nc.tensor.matmul(wps[:], wrm[:, 0:128], wrm[:],
                                     start=True, stop=True)

            cos_sb = cpool.tile([DH, t], BF16, tag="cos")
            sin_sb = cpool.tile([DH, t], BF16, tag="sin")
            bm_sb = cpool.tile([128, 128], BF16, tag="bm")
            id_sb = cpool.tile([128, 128], BF16, tag="id")
            rot_sb = cpool.tile([128, 128], BF16, tag="rot")
            # fp8 hi/lo attention outputs, head-plane layout for DoubleRow
            oTh_sb = opool.tile([128, HPC, t], FP8, tag="oTh", name="oTh")
            oTl_sb = opool.tile([128, HPC, t], FP8, tag="oTl", name="oTl")
            wph_sb = wpj.tile([128, HPC, D], FP8, tag="wph", name="wph")
            wpl_sb = wpj.tile([128, HPC, D], FP8, tag="wpl", name="wpl")

            def rope_ip(qk, tb, name, eng=None):
                """In-place RoPE on qk[:, tb-block] (holds the WS-scaled raw
                projection).  cos/sin are host-scaled by 1/WS, so the result
                is the true-scale rotated q/k.  The half-rotation runs on the
                PE as a signed permutation matmul (own psum pool so it never
                steals a qkv/score bank).  `eng` picks the engine for the two
                elementwise muls: gpsimd for latency-tolerant in-loop ropes,
                vector for phase-boundary ones."""
                t0, t1_ = TBE * tb, TBE * (tb + 1)
                blk = qk[:, t0:t1_]
                # pre-sin form: rot(blk*sin) == rot(blk)*sin because RoPE's
                # emb duplicates the frequencies in both halves (sin[d] ==
                # sin[(d+64)%128]).  This leaves only one DVE op (the psum
                # add) after the PE rotation instead of two.
                if eng is None:
                    eng = nc.vector
                m2, r1 = rope_mul(qk, tb, name, eng)
                rope_fin(qk, tb, name, m2, r1)

            def rope_mul(qk, tb, name, eng=None):
                """The two elementwise rope muls — only depend on the parked
                q/k block, so they can issue right after the park while the
                rot+add defer to the next schedule slot."""
                t0, t1_ = TBE * tb, TBE * (tb + 1)
                blk = qk[:, t0:t1_]
                if eng is None:
                    eng = nc.vector
                m2 = tmpp.tile([128, TBE], BF16, tag="r2", name=f"m2_{name}")
                eng.tensor_mul(m2[:], blk, sin_sb[:, t0:t1_])
                r1 = tmpp.tile([128, TBE], BF16, tag="r1", name=f"r1_{name}")
                eng.tensor_mul(r1[:], blk, cos_sb[:, t0:t1_])
                return m2, r1

            def rope_fin(qk, tb, name, m2, r1):
                t0, t1_ = TBE * tb, TBE * (tb + 1)
                blk = qk[:, t0:t1_]
                rps = ps_rp.tile([128, TBE], F32, tag="rp", name=f"rot_{name}")
                nc.tensor.matmul(rps[:], rot_sb[:], m2[:], start=True, stop=True)
                nc.vector.tensor_add(blk, r1[:], rps[:])

            def emit_proj(tt, on_act=False, tail=False):
                """y[128*tt:128*(tt+1), :] = sum_h oT_h[:, tt].T @ wp_h, as
                3-term fp8 DoubleRow over head pairs.  Psum carries WS*y;
                the drains (split across Act/DVE) fold in 1/WS.  One DMA per
                row-block normally; per-512-chunk DMAs on the tail so the
                last transfer isn't a serial 1MB copy."""
                yst = ysp.tile([128, D], BF16, tag="yst", name=f"yst{tt}")
                for db in range(D // 512):
                    ps = ps_mm.tile([128, 512], F32, tag="mm", name=f"psy{tt}_{db}")
                    idx = 0
                    for lhs_t, rhs_t in ((oTh_sb, wph_sb), (oTl_sb, wph_sb), (oTh_sb, wpl_sb)):
                        for g in range(HPC // 2):
                            nc.tensor.matmul(
                                ps[:],
                                lhs_t[:, 2 * g:2 * g + 2, 128 * tt:128 * (tt + 1)],
                                rhs_t[:, 2 * g:2 * g + 2, 512 * db:512 * (db + 1)],
                                start=(idx == 0), stop=(idx == 3 * (HPC // 2) - 1),
                                perf_mode=DR)
                            idx += 1
                    ys = yst[:, 512 * db:512 * (db + 1)]
                    if tail and db % 2 == (0 if on_act else 1):
                        nc.scalar.mul(ys, ps[:], 1.0 / WS)
                    else:
                        # keep the Act queue free for exps: non-tail drains
                        # all run on DVE
                        nc.vector.tensor_scalar_mul(ys, ps[:], 1.0 / WS)
                    if tail:
                        nc.sync.dma_start(
                            y[128 * tt:128 * (tt + 1), 512 * db:512 * (db + 1)], ys)
                if not tail:
                    nc.sync.dma_start(y[128 * tt:128 * (tt + 1), :], yst[:])

            pair_bufs = {}
            pend_ropes = []

            def qkv_pair_steps(p2):
                """Generator: pair-p2 weight/x DMAs + fp8 qkv projection;
                yields once per t-block."""
                h = 2 * p2
                q_sbs = [None, None]
                k_sbs = [None, None]
                vext = [None, None]
                # fp8 weight tiles, [128, ND, 128]-shaped for DoubleRow pairs
                wq = [[wqkp.tile([128, ND, 128], FP8, tag=f"wq{i}{lv}", name=f"wq{i}{lv}_{p2}")
                       for lv in range(2)] for i in range(2)]
                wk = [[wqkp.tile([128, ND, 128], FP8, tag=f"wk{i}{lv}", name=f"wk{i}{lv}_{p2}")
                       for lv in range(2)] for i in range(2)]
                wv = [wvp.tile([128, ND, 256], FP8, tag=f"wv{lv}", name=f"wv{lv}_{p2}")
                      for lv in range(2)]
                vext[0] = vep.tile([128, NT, 129], BF16, tag="ve0", name=f"ve0_{p2}")
                vext[1] = vep.tile([128, NT, 129], BF16, tag="ve1", name=f"ve1_{p2}")
                # only the ones-columns need init (v-parks overwrite the rest);
                # 64.0 folds the WS weight scale out of the normalizer.
                nc.vector.memset(vext[0][:, :, 128:129], WS)
                nc.vector.memset(vext[1][:, :, 128:129], WS)
                q_sbs[0] = qkpool.tile([DH, t], BF16, tag="q0", name=f"q0_{p2}")
                k_sbs[0] = qkpool.tile([DH, t], BF16, tag="k0", name=f"k0_{p2}")
                q_sbs[1] = qkpool.tile([DH, t], BF16, tag="q1", name=f"q1_{p2}")
                k_sbs[1] = qkpool.tile([DH, t], BF16, tag="k1", name=f"k1_{p2}")
                pair_bufs[p2] = {"q": q_sbs, "k": k_sbs, "ve": vext}

                # x tiles for this pair (double-buffered across t-blocks)
                xts = {}
                loop_ropes = []

                def dma_x(tb):
                    xh_t = xtp.tile([128, ND, TBE], FP8, tag="xh", name=f"xh{p2}_{tb}")
                    xl_t = xtp.tile([128, ND, TBE], FP8, tag="xl", name=f"xl{p2}_{tb}")
                    t0, t1_ = TBE * tb, TBE * (tb + 1)
                    hd = ND // 2
                    for hf in range(2):
                        nc.sync.dma_start(
                            xh_t[:, hd * hf:hd * (hf + 1), :],
                            xTh[1024 * hf:1024 * (hf + 1), t0:t1_].rearrange(
                                "(j p) c -> p j c", p=128))
                    for hf in range(2):
                        nc.sync.dma_start(
                            xl_t[:, hd * hf:hd * (hf + 1), :],
                            xTl[1024 * hf:1024 * (hf + 1), t0:t1_].rearrange(
                                "(j p) c -> p j c", p=128))
                    xts[tb] = (xh_t, xl_t)

                # DMA order tuned so the first q0 matmul starts ~3us in and
                # every later consumer arrives just ahead of its first use:
                # wq0h, x-hi, wk0h, wq1h/wk1h, x-lo, lo-weights, wv.
                w0 = D * h
                w1 = D * (h + 1)
                xh_t = xtp.tile([128, ND, TBE], FP8, tag="xh", name=f"xh{p2}_0")
                xl_t = xtp.tile([128, ND, TBE], FP8, tag="xl", name=f"xl{p2}_0")
                hd = ND // 2
                qd = ND // 4
                nc.sync.dma_start(wq[0][0][:], wqh_hi[:, w0:w1])
                nc.sync.dma_start(
                    xh_t[:, 0:qd, :],
                    xTh[0:512, 0:TBE].rearrange("(j p) c -> p j c", p=128))
                nc.sync.dma_start(
                    xh_t[:, qd:2 * qd, :],
                    xTh[512:1024, 0:TBE].rearrange("(j p) c -> p j c", p=128))
                nc.sync.dma_start(wk[0][0][:], wkh_hi[:, w0:w1])
                nc.sync.dma_start(
                    xh_t[:, 2 * qd:3 * qd, :],
                    xTh[1024:1536, 0:TBE].rearrange("(j p) c -> p j c", p=128))
                nc.sync.dma_start(
                    xh_t[:, 3 * qd:ND, :],
                    xTh[1536:2048, 0:TBE].rearrange("(j p) c -> p j c", p=128))
                nc.sync.dma_start(wq[1][0][:], wqh_hi[:, w1:w1 + D])
                nc.sync.dma_start(wk[1][0][:], wkh_hi[:, w1:w1 + D])
                # term-1 lo weights arrive before term-2's x-lo
                nc.sync.dma_start(wq[0][1][:], wqh_lo[:, w0:w1])
                nc.sync.dma_start(wk[0][1][:], wkh_lo[:, w0:w1])
                nc.sync.dma_start(wq[1][1][:], wqh_lo[:, w1:w1 + D])
                nc.sync.dma_start(wk[1][1][:], wkh_lo[:, w1:w1 + D])
                for hf in range(2):
                    nc.sync.dma_start(
                        xl_t[:, hd * hf:hd * (hf + 1), :],
                        xTl[1024 * hf:1024 * (hf + 1), 0:TBE].rearrange(
                            "(j p) c -> p j c", p=128))
                xts[0] = (xh_t, xl_t)
                if p2 == 0:
                    # only tb0's cos/sin chunk up front; later chunks stream
                    # per-t-block so the x prefetches aren't queued behind
                    # 1MB of consts on the serial DMA device
                    nc.sync.dma_start(cos_sb[:, 0:TBE], cosT[:, 0:TBE])
                    nc.sync.dma_start(sin_sb[:, 0:TBE], sinTm[:, 0:TBE])
                nc.sync.dma_start(wv[0][:], wvh_hi[:, 2 * D * p2:2 * D * (p2 + 1)])
                nc.sync.dma_start(wv[1][:], wvh_lo[:, 2 * D * p2:2 * D * (p2 + 1)])
                if p2 == 0:
                    nc.sync.dma_start(rot_sb[:], rotm[:])
                    nc.sync.dma_start(bm_sb[:], bmask[:])
                    nc.sync.dma_start(id_sb[:], ident[:])
                else:
                    # prefetch the projection weights during pair-1 qkv
                    for hh in range(HPC):
                        nc.sync.dma_start(
                            wph_sb[:, hh, :], wpT_hi[128 * hh:128 * (hh + 1), :])
                        nc.sync.dma_start(
                            wpl_sb[:, hh, :], wpT_lo[128 * hh:128 * (hh + 1), :])

                for tb in range(NTB):
                    t0, t1_ = TBE * tb, TBE * (tb + 1)
                    for qk, tb_, nm_, m2_, r1_ in loop_ropes:
                        rope_fin(qk, tb_, nm_, m2_, r1_)
                    loop_ropes.clear()
                    if tb + 1 < NTB:
                        dma_x(tb + 1)
                        if p2 == 0:
                            nt0, nt1 = TBE * (tb + 1), TBE * (tb + 2)
                            nc.sync.dma_start(cos_sb[:, nt0:nt1], cosT[:, nt0:nt1])
                            nc.sync.dma_start(sin_sb[:, nt0:nt1], sinTm[:, nt0:nt1])
                    xh_t, xl_t = xts.pop(tb)
                    ps_q0 = ps_mm.tile([128, TBE], F32, tag="mm", name=f"psq0_{p2}_{tb}")
                    ps_k0 = ps_mm.tile([128, TBE], F32, tag="mm", name=f"psk0_{p2}_{tb}")
                    ps_q1 = ps_mm.tile([128, TBE], F32, tag="mm", name=f"psq1_{p2}_{tb}")
                    ps_k1 = ps_mm.tile([128, TBE], F32, tag="mm", name=f"psk1_{p2}_{tb}")
                    nvp = (TTPB + 1) // 2
                    ps_vs = [
                        ps_sm.tile([128, 512], F32, tag="sm", name=f"psv{p2}_{tb}_{i}")
                        for i in range(nvp)
                    ]
                    # 3-term fp8 accumulation: wh*xh + wl*xh + wh*xl,
                    # term-major (matches the tb-0 DMA arrival order); each
                    # psum parks right after its last matmul so the bank
                    # frees and the copy overlaps the remaining matmuls.
                    last_tb = tb == NTB - 1
                    qk_psums = ((ps_q0, wq[0], q_sbs[0]), (ps_k0, wk[0], k_sbs[0]),
                                (ps_q1, wq[1], q_sbs[1]), (ps_k1, wk[1], k_sbs[1]))

                    def qk_sweep():
                        for term in range(3):
                            wlv = 1 if term == 1 else 0
                            xt = xl_t if term == 2 else xh_t
                            first = term == 0
                            last = term == 2
                            for i_, (ps, wt, dst) in enumerate(qk_psums):
                                for j in range(NJ):
                                    js = slice(2 * j, 2 * j + 2)
                                    nc.tensor.matmul(ps[:], wt[wlv][:, js, :], xt[:, js, :],
                                                     start=(first and j == 0),
                                                     stop=(last and j == NJ - 1),
                                                     perf_mode=DR)
                                if last:
                                    # split parks across Act/DVE once
                                    # attention exps share the Act queue
                                    if tb > 0 and i_ % 2 == 1:
                                        nc.vector.tensor_copy(dst[:, t0:t1_], ps[:])
                                    else:
                                        nc.scalar.copy(dst[:, t0:t1_], ps[:])

                    def v_sweep():
                        for term in range(3):
                            wvt = wv[1 if term == 1 else 0]
                            xt = xl_t if term == 2 else xh_t
                            first = term == 0
                            last = term == 2
                            for j in range(NJ):
                                js = slice(2 * j, 2 * j + 2)
                                for tt in range(TTPB):
                                    nc.tensor.matmul(
                                        ps_vs[tt // 2][:, 256 * (tt % 2):256 * (tt % 2) + 256],
                                        xt[:, js, 128 * tt:128 * (tt + 1)],
                                        wvt[:, js, :],
                                        start=(first and j == 0 and tt % 2 == 0),
                                        stop=(last and j == NJ - 1),
                                        skip_group_check=True, perf_mode=DR)
                        for tt in range(TTPB):
                            gt = tb * TTPB + tt
                            o0 = 256 * (tt % 2)
                            vc = nc.vector.tensor_copy if (tb > 0 and tt % 2) else nc.scalar.copy
                            vc(vext[0][:, gt, 0:128], ps_vs[tt // 2][:, o0:o0 + 128])
                            vc(vext[1][:, gt, 0:128], ps_vs[tt // 2][:, o0 + 128:o0 + 256])

                    # last t-block: V first so its psums drain under the q/k
                    # matmuls and the pair boundary ends on parks+ropes only
                    if last_tb and tb > 0:
                        v_sweep()
                        qk_sweep()
                    else:
                        qk_sweep()
                        v_sweep()
                    # RoPE policy: in-loop ropes are queued and ISSUED at the
                    # start of the next t-block slot, so their rot matmuls
                    # land ahead of the 15us qkv stream in the in-order PE
                    # queue (a rope add stuck behind a whole t-block blocks
                    # every later DVE op).  Pair 0 rotates all four heads
                    # in-loop; pair 1 only its h2 head (h3's rotate in phase
                    # 3 via scheduler-placed consume_ropes).  The last
                    # t-block always defers.
                    if tb != NTB - 1:
                        heads = [0, 1] if p2 == 1 else [0]
                        for par in heads:
                            for qk, pfx in ((q_sbs[par], "q"), (k_sbs[par], "k")):
                                nm = f"{pfx}{par}_{p2}_{tb}"
                                m2_, r1_ = rope_mul(qk, tb, nm, eng=nc.gpsimd)
                                loop_ropes.append((qk, tb, nm, m2_, r1_))
                        if p2 == 0:
                            pend_ropes.append((q_sbs[1], tb, f"q1_{p2}_{tb}"))
                            pend_ropes.append((k_sbs[1], tb, f"k1_{p2}_{tb}"))
                    else:
                        # the pair's own (h-even) tail ropes go to the FRONT
                        # so schedule-placed consume_ropes() can rotate them
                        # before the partner-head ones
                        pend_ropes.insert(0, (k_sbs[0], tb, f"k0_{p2}_{tb}"))
                        pend_ropes.insert(0, (q_sbs[0], tb, f"q0_{p2}_{tb}"))
                        pend_ropes.append((q_sbs[1], tb, f"q1_{p2}_{tb}"))
                        pend_ropes.append((k_sbs[1], tb, f"k1_{p2}_{tb}"))
                    yield

            def attention_steps(h, quotas, pipelined, do_proj=False):
                """Generator for head h's attention, yielding once per chunk."""
                par = h % 2
                bufs = pair_bufs[h // 2]
                q_sb, k_sb = bufs["q"][par], bufs["k"][par]
                ve = bufs["ve"][par]
                if par == 0 and pend_ropes:
                    mine = [e for e in pend_ropes if e[0] is q_sb or e[0] is k_sb]
                    rest = [e for e in pend_ropes if not (e[0] is q_sb or e[0] is k_sb)]
                    pend_ropes[:] = mine + rest

                def stage_a(ib):
                    """scores + exp + diagonal mask for i-block ib; the score
                    matmul and exp narrow to the causal range on diagonal
                    j-tiles."""
                    i0 = sb * ib
                    jt_max = (i0 + sb) // 128 - 1  # inclusive
                    pts = [None] * (jt_max + 1)
                    for jt in range(jt_max + 1):
                        m = jt - NIC * ib
                        off = 128 * m if m > 0 else 0
                        s_ps = ps_mm.tile([128, sb], F32, tag="mm", name=f"s{h}_{ib}_{jt}")
                        nc.tensor.matmul(
                            s_ps[:, off:sb],
                            k_sb[:, 128 * jt:128 * (jt + 1)],
                            q_sb[:, i0 + off:i0 + sb],
                            start=True, stop=True)
                        pt_t = ptp.tile([128, sb], BF16, tag="pt", name=f"pt{h}_{ib}_{jt}")
                        nc.scalar.activation(pt_t[:, off:sb], s_ps[:, off:sb], AF.Exp, scale=SCALE)
                        if m >= 0:
                            pm = pt_t[:, 128 * m:128 * (m + 1)]
                            nc.vector.tensor_mul(pm, pm, bm_sb[:])
                        pts[jt] = pt_t
                    return pts

                def stage_b(ib, pts, bi):
                    """PV + normalize + transpose + fp8 hi/lo split."""
                    i0 = sb * ib

                    def finish(ic, pv):
                        rc = smallp.tile([128, 1], F32, tag="rc", name=f"rc{h}_{ib}_{ic}")
                        nc.vector.reciprocal(rc[:], pv[:, 128:129])
                        o_sb = smallp.tile([128, 128], BF16, tag="o", name=f"o{h}_{ib}_{ic}")
                        nc.vector.tensor_scalar_mul(o_sb[:], pv[:, 0:128], rc[:])
                        ot_ps = ps_mm.tile([128, 128], BF16, tag="mm", name=f"otp{h}_{ib}_{ic}")
                        nc.tensor.transpose(ot_ps[:], o_sb[:], id_sb[:])
                        c0 = i0 + 128 * ic
                        hs = oTh_sb[:, h, c0:c0 + 128]
                        if do_proj:
                            # proj-critical head: split straight off the psum
                            # on Act + DVE (no Pool latency in the chain)
                            nc.scalar.copy(hs, ot_ps[:])
                            nc.vector.scalar_tensor_tensor(
                                oTl_sb[:, h, c0:c0 + 128], hs, -1.0, ot_ps[:],
                                mybir.AluOpType.mult, mybir.AluOpType.add)
                        else:
                            otb = smallp.tile([128, 128], BF16, tag="otb", name=f"otb{h}_{ib}_{ic}")
                            nc.vector.tensor_copy(otb[:], ot_ps[:])
                            # fp8 hi/lo split on GpSimd (SBUF-only engine)
                            nc.gpsimd.tensor_copy(hs, otb[:])
                            nc.gpsimd.tensor_sub(oTl_sb[:, h, c0:c0 + 128], otb[:], hs)

                    prev = None
                    for ic in range(NIC):
                        last_jt = NIC * ib + ic
                        pv = ps_sm.tile([128, 129], F32, tag="sm", name=f"pv{h}_{ib}_{ic}")
                        for jt in range(last_jt + 1):
                            nc.tensor.matmul(
                                pv[:],
                                pts[jt][:, 128 * ic:128 * (ic + 1)],
                                ve[:, jt, :],
                                start=(jt == 0), stop=(jt == last_jt))
                        if prev is not None:
                            finish(*prev)
                            if do_proj:
                                emit_proj(NIC * ib + prev[0], on_act=False,
                                          tail=(ib == NSB - 1))
                        prev = (ic, pv)
                    finish(*prev)
                    if do_proj:
                        emit_proj(NIC * ib + prev[0],
                                  on_act=(ib == NSB - 1), tail=(ib == NSB - 1))
                    consume_ropes(quotas[bi])

                if pipelined:
                    pts_prev = None
                    for ib in range(NSB):
                        pts_cur = stage_a(ib)
                        yield
                        if pts_prev is not None:
                            stage_b(ib - 1, pts_prev, ib - 1)
                            yield
                        pts_prev = pts_cur
                    stage_b(NSB - 1, pts_prev, NSB - 1)
                    yield
                else:
                    for ib in range(NSB):
                        pts = stage_a(ib)
                        yield
                        stage_b(ib, pts, ib)
                        yield

            def stepn(g, n):
                for _ in range(n):
                    next(g)

            def run(g):
                for _ in g:
                    pass

            def consume_ropes(n):
                for qk, tb_, nm_ in pend_ropes[:n]:
                    rope_ip(qk, tb_, nm_)
                del pend_ropes[:n]

            # ---- schedule -------------------------------------------
            # pair 0 qkv alone; h0/h1 attention chunks (mutually interleaved
            # so one head's scores fill the other's exp latency) interleaved
            # with pair 1's qkv t-blocks; h2/h3 likewise interleaved with
            # each other and the output projection folded into h3's B chunks.
            if NTB >= 4 and NSB >= 4:
                Z = [0, 0, 0, 0]
                # h0's attention starts one i-block behind pair-0's qkv
                # t-blocks (its k/v prefix is complete by then), filling
                # pair-0's otherwise idle Act with exps; deferred ropes are
                # consumed at explicit schedule points, each before any chunk
                # that reads the rotated tile.
                q0 = qkv_pair_steps(0)
                a0 = attention_steps(0, Z, True)
                stepn(q0, 2)        # tb0 tb1 (tb0's ropes issue at tb1 start)
                stepn(a0, 1)        # A0
                stepn(q0, 1)        # tb2 (ropes tb1)
                stepn(a0, 2)        # A1 B0
                stepn(q0, 1)        # tb3 (ropes tb2)
                consume_ropes(2)    # q0/k0 pair0-tb3
                stepn(a0, 3)        # A2 B1 A3
                consume_ropes(4)    # q1/k1 pair0-tb0, tb1
                a1 = attention_steps(1, Z, True)
                q1 = qkv_pair_steps(1)
                a2 = attention_steps(2, Z, True)
                stepn(a0, 1)        # B2
                stepn(a1, 1)        # A0
                stepn(q1, 1)        # pair1 tb0
                consume_ropes(2)    # q1/k1 pair0-tb2
                stepn(a1, 2)        # A1 B0
                stepn(a0, 1)        # B3
                stepn(q1, 1)        # tb1 (ropes p1-tb0)
                consume_ropes(2)    # q1/k1 pair0-tb3
                stepn(a2, 1)        # h2.A0
                stepn(a1, 2)        # A2 B1
                stepn(q1, 1)        # tb2 (ropes p1-tb1)
                stepn(a2, 2)        # h2.A1 B0
                stepn(a1, 2)        # A3 B2
                stepn(q1, 1)        # tb3 (ropes p1-tb2)
                consume_ropes(2)    # q0/k0 pair1-tb3
                stepn(a2, 2)        # h2.A2 B1
                run(a1)             # B3
                run(a0)
                run(q0)
                run(q1)
                # h3's first chunks (and ib0's projection) pull into the
                # phase-2 tail where Act still has headroom; phase 3 is the
                # remainder with h2's tail interleaved.
                a3 = attention_steps(3, Z, True, do_proj=True)
                consume_ropes(2)    # q1/k1 pair1-tb3
                stepn(a3, 3)        # A0 A1 B0 (+proj ib0)
                stepn(a2, 1)        # h2.A3
                stepn(a2, 1)        # h2.B2
                stepn(a3, 2)        # A2 B1 (+proj ib1)
                stepn(a2, 1)        # h2.B3
                stepn(a3, 1)        # A3
                run(a3)             # B2 B3 (+proj ib2, ib3)
                run(a2)
            else:
                run(qkv_pair_steps(0))
                run(attention_steps(0, [2, 2, 0, 0], True))
                run(attention_steps(1, [0, 0, 0, 0], True))
                run(qkv_pair_steps(1))
                run(attention_steps(2, [2, 2, 0, 0], True))
                run(attention_steps(3, [0, 0, 0, 0], True, do_proj=True))

    nc.compile()
    return nc


def host_consts(t=T):
    """RoPE cos/sin (scaled by 1/WS to fold out the fp8 weight pre-scale),
    causal big-mask, identity, signed half-rotation."""
    inv = (1.0 / (np.float32(10000.0) ** (np.arange(0, DH, 2, dtype=np.float32) / np.float32(DH)))).astype(np.float32)
    tt = np.arange(t, dtype=np.float32)
    fr = np.outer(tt, inv).astype(np.float32)       # [t, 64]
    emb = np.concatenate([fr, fr], axis=1)          # [t, 128]
    cosT = np.ascontiguousarray(np.cos(emb).T.astype(np.float32)) / np.float32(WS)
    sinTm = np.ascontiguousarray(np.sin(emb).T.astype(np.float32)) / np.float32(WS)
    jj = np.arange(128)[:, None]
    cc = np.arange(128)[None, :]
    bmask = (cc >= jj).astype(np.float32)
    ident = np.eye(128, dtype=np.float32)
    # signed half-rotation: (rotm.T @ x)[d] = -x[d+64] for d<64, x[d-64] else
    rotm = np.zeros((128, 128), dtype=np.float32)
    for d in range(64):
        rotm[d + 64, d] = -1.0
        rotm[d, d + 64] = 1.0
    return cosT, sinTm, bmask, ident, rotm


def _warrange(w):
    """[128*nh rows, D] head-major weight slice -> [128, nh*D] sbuf-ready layout:
    block h, col di*128+c of partition p  =  w[128*h + c, 128*di + p]."""
    nh = w.shape[0] // 128
    d = w.shape[1]
    out = np.empty((128, nh * d), dtype=w.dtype)
    for h in range(nh):
        a = w[128 * h:128 * (h + 1), :].T.reshape(d // 128, 128, 128)  # [di, p, c]
        out[:, d * h:d * (h + 1)] = a.transpose(1, 0, 2).reshape(128, d)
    return out


def _wvarrange(w):
    """[512 rows, D] 4-head v-weights -> [128, 2*2*D]: per pair, di-major blocks of
    [even-head 128 cols | odd-head 128 cols]."""
    d = w.shape[1]
    blocks = []
    for p2 in range(2):
        e = w[256 * p2:256 * p2 + 128, :].T.reshape(d // 128, 128, 128)
        o = w[256 * p2 + 128:256 * p2 + 256, :].T.reshape(d // 128, 128, 128)
        pair = np.concatenate([e, o], axis=2)          # [di, p, 256]
        blocks.append(pair.transpose(1, 0, 2).reshape(128, 2 * d))
    return np.concatenate(blocks, axis=1)


FP8NP = ml_dtypes.float8_e4m3


def _split8(a):
    """Error-compensated e4m3 split: a ~= hi + lo (fp32 in, fp8 out pair)."""
    a = np.asarray(a, dtype=np.float32)
    hi = a.astype(FP8NP)
    lo = (a - hi.astype(np.float32)).astype(FP8NP)
    return hi, lo


def shard_inputs(x, w_qkv, w_proj, t=T, pv_dt="bfloat16"):
    """Build the 8 per-core input maps (host does the fp8 splits)."""
    bdt = ml_dtypes.bfloat16
    cosT, sinTm, bmask, ident, rotm = host_consts(t)
    cosT = cosT.astype(bdt)
    sinTm = sinTm.astype(bdt)
    bmask = bmask.astype(bdt)
    ident = ident.astype(bdt)
    rotm = rotm.astype(bdt)
    d = x.shape[2]
    ws = np.float32(WS)
    in_maps = []
    xs = {}
    for b in range(x.shape[0]):
        xs[b] = _split8(np.ascontiguousarray(x[b].T))
    for c in range(8):
        b, g = divmod(c, 4)
        s0, s1 = 512 * g, 512 * (g + 1)
        wq_hi, wq_lo = _split8(w_qkv[s0:s1, :] * ws)
        wk_hi, wk_lo = _split8(w_qkv[d + s0:d + s1, :] * ws)
        wv_hi, wv_lo = _split8(w_qkv[2 * d + s0:2 * d + s1, :] * ws)
        wp_hi, wp_lo = _split8(np.ascontiguousarray(w_proj[:, s0:s1].T) * ws)
        in_maps.append(dict(
            xTh=xs[b][0], xTl=xs[b][1],
            wqh_hi=_warrange(wq_hi), wqh_lo=_warrange(wq_lo),
            wkh_hi=_warrange(wk_hi), wkh_lo=_warrange(wk_lo),
            wvh_hi=_wvarrange(wv_hi), wvh_lo=_wvarrange(wv_lo),
            wpT_hi=wp_hi, wpT_lo=wp_lo,
            cosT=cosT, sinTm=sinTm, bmask=bmask, ident=ident, rotm=rotm,
        ))
    return in_maps


_NC_CACHE = {}


def get_nc(t=T, mm_dt="float32r", pv_dt="bfloat16"):
    key = (t, mm_dt, pv_dt)
    if key not in _NC_CACHE:
        _NC_CACHE[key] = build_nc(t=t, mm_dt=mm_dt, pv_dt=pv_dt)
    return _NC_CACHE[key]


def kernel(x, w_qkv, w_proj):
    x = np.asarray(x, dtype=np.float32)
    w_qkv = np.asarray(w_qkv, dtype=np.float32)
    w_proj = np.asarray(w_proj, dtype=np.float32)
    b_, t_, d_ = x.shape
    in_maps = shard_inputs(x, w_qkv, w_proj, t=t_)
    nc = get_nc(t=t_)
    res = run_bass_kernel_spmd(nc, in_maps, list(range(8))).results
    out = np.zeros((b_, t_, d_), dtype=np.float32)
    for c in range(8):
        b, _ = divmod(c, 4)
        out[b] += res[c]["y"]
    return out


# revision 68
# speedup vs baseline: 1.0731x; 1.0034x over previous
"""Trainium2 Bass kernel: causal self-attention with RoPE (B=2, T=2048, D=2048, H=16).

Sharding: 8 cores = 2-way data parallel over batch x 4-way tensor parallel over
heads.  Core c = 4*b + g computes batch b, heads 4g..4g+3, and produces a
partial output y_partial = attn_out[:, heads_g] @ w_proj[:, heads_g].T which the
host sums over g.

Per-core pipeline (v3 — fp8 DoubleRow for the projections):
  - qkv projection runs in fp8e4 with MatmulPerfMode.DoubleRow (2 K-planes per
    instruction at 0.5 cycles/row) using a host-side error-compensated split:
    w = wh + wl, x = xh + xl (each term an e4m3 tensor), accumulating
    wh*xh + wl*xh + wh*xl into the same psum (12N cycles per 128x2048-contract
    tile vs 16N in fp32r/bf16).  Weights are pre-scaled by 2^6 on the host so
    the hi/lo parts stay out of e4m3's subnormal floor; the 2^-6 is folded
    into the (host-precomputed) rope cos/sin for q/k, into the ones-columns of
    the V staging tiles (memset 64.0: den = 64*sum(p), o = num*(1/den)*64
    cancels), and into the final y drain (scaled copy) for the projection.
  - scores/PV stay bf16; score matmuls + exp narrow to the causal range on the
    diagonal j-tiles (the skipped top-left pt region is never read).
  - attention output oT is split to fp8 hi/lo on the (otherwise idle) GpSimd
    engine; the output projection is again 3-term DoubleRow over head pairs.
  - issue-level interleave: pair-1 qkv t-blocks are issued between h0/h1
    attention chunks so qkv matmuls fill the exp-latency PE gaps.
"""

import sys

import numpy as np
import ml_dtypes

for _p in ("/opt/trn_rl_repo", "/root/.axon_site/_ro/trn_rl_repo"):
    if _p not in sys.path:
        sys.path.append(_p)

import concourse.bass as bass
import concourse.bacc as bacc
import concourse.tile as tile
from concourse import mybir
from concourse.bass_utils import run_bass_kernel_spmd

F32 = mybir.dt.float32
BF16 = mybir.dt.bfloat16
FP8 = mybir.dt.float8e4
AF = mybir.ActivationFunctionType
DR = mybir.MatmulPerfMode.DoubleRow

B, T, D, H = 2, 2048, 2048, 16
HPC = H // 4  # heads per core (4-way head TP)
DH = D // H   # 128
SCALE = float(DH) ** -0.5
WS = 64.0     # host-side weight pre-scale (2^6)

TB = 512      # qkv-projection t-block (psum free width)
SB = 512      # attention i-block (score free width)


def build_nc(t=T, mm_dt="float32r", pv_dt="bfloat16"):
    """Build the SPMD per-core program.  `t` is the sequence length (smaller
    values are used for simulator validation).  mm_dt/pv_dt kept for test.py
    compatibility (ignored: matmuls are fp8-DoubleRow / bf16)."""
    NT = t // 128    # token tiles
    TBE = min(TB, t)
    NTB = t // TBE   # qkv t-blocks
    sb = min(SB, t)
    NSB = t // sb    # attention i-blocks
    NIC = sb // 128  # i-chunks per i-block
    ND = D // 128    # contraction d-tiles
    NJ = ND // 2     # DoubleRow K-pair count for the qkv contraction
    TTPB = TBE // 128

    nc = bacc.Bacc("TRN2", target_bir_lowering=False, debug=False)

    xTh = nc.dram_tensor("xTh", [D, t], FP8, kind="ExternalInput").ap()
    xTl = nc.dram_tensor("xTl", [D, t], FP8, kind="ExternalInput").ap()
    wqh_hi = nc.dram_tensor("wqh_hi", [128, HPC * D], FP8, kind="ExternalInput").ap()
    wqh_lo = nc.dram_tensor("wqh_lo", [128, HPC * D], FP8, kind="ExternalInput").ap()
    wkh_hi = nc.dram_tensor("wkh_hi", [128, HPC * D], FP8, kind="ExternalInput").ap()
    wkh_lo = nc.dram_tensor("wkh_lo", [128, HPC * D], FP8, kind="ExternalInput").ap()
    wvh_hi = nc.dram_tensor("wvh_hi", [128, (HPC // 2) * 2 * D], FP8, kind="ExternalInput").ap()
    wvh_lo = nc.dram_tensor("wvh_lo", [128, (HPC // 2) * 2 * D], FP8, kind="ExternalInput").ap()
    wpT_hi = nc.dram_tensor("wpT_hi", [HPC * DH, D], FP8, kind="ExternalInput").ap()
    wpT_lo = nc.dram_tensor("wpT_lo", [HPC * DH, D], FP8, kind="ExternalInput").ap()
    cosT = nc.dram_tensor("cosT", [DH, t], BF16, kind="ExternalInput").ap()
    sinTm = nc.dram_tensor("sinTm", [DH, t], BF16, kind="ExternalInput").ap()
    bmask = nc.dram_tensor("bmask", [128, 128], BF16, kind="ExternalInput").ap()
    ident = nc.dram_tensor("ident", [128, 128], BF16, kind="ExternalInput").ap()
    rotm = nc.dram_tensor("rotm", [128, 128], BF16, kind="ExternalInput").ap()
    y = nc.dram_tensor("y", [t, D], BF16, kind="ExternalOutput").ap()

    with tile.TileContext(nc) as tc:
        with (
            tc.tile_pool(name="consts", bufs=1) as cpool,
            tc.tile_pool(name="oTp", bufs=1) as opool,
            tc.tile_pool(name="qkp", bufs=2) as qkpool,
            tc.tile_pool(name="xtp", bufs=2) as xtp,
            tc.tile_pool(name="wqkp", bufs=1) as wqkp,
            tc.tile_pool(name="wvp", bufs=1) as wvp,
            tc.tile_pool(name="wpj", bufs=1) as wpj,
            tc.tile_pool(name="vep", bufs=2) as vep,
            tc.tile_pool(name="ptp", bufs=max(2 * NT + 4, NT + 1)) as ptp,
            tc.tile_pool(name="tmpp", bufs=6) as tmpp,
            tc.tile_pool(name="smallp", bufs=8) as smallp,
            tc.tile_pool(name="ysp", bufs=2) as ysp,
            tc.tile_pool(name="ps_mm", bufs=5, space="PSUM") as ps_mm,
            tc.tile_pool(name="ps_sm", bufs=2, space="PSUM") as ps_sm,
            tc.tile_pool(name="ps_rp", bufs=1, space="PSUM") as ps_rp,
        ):
            # PE warmup: dummy matmuls on a memset tile bridge the initial
            # DMA wait so the p-state ramp (half-rate for 3us after idle)
            # completes before the first real matmul.
            if t >= 2048:
                wrm = cpool.tile([128, 512], BF16, tag="wrm", name="wrm")
                nc.vector.memset(wrm[:], 0.0)
                for wi in range(10):
                    wps = ps_rp.tile([128, 512], F32, tag="rp", name=f"warm{wi}")
                    nc.tensor.matmul(wps[:], wrm[:, 0:128], wrm[:],
                                     start=True, stop=True)

            cos_sb = cpool.tile([DH, t], BF16, tag="cos")
            sin_sb = cpool.tile([DH, t], BF16, tag="sin")
            bm_sb = cpool.tile([128, 128], BF16, tag="bm")
            id_sb = cpool.tile([128, 128], BF16, tag="id")
            rot_sb = cpool.tile([128, 128], BF16, tag="rot")
            # fp8 hi/lo attention outputs, head-plane layout for DoubleRow
            oTh_sb = opool.tile([128, HPC, t], FP8, tag="oTh", name="oTh")
            oTl_sb = opool.tile([128, HPC, t], FP8, tag="oTl", name="oTl")
            wph_sb = wpj.tile([128, HPC, D], FP8, tag="wph", name="wph")
            wpl_sb = wpj.tile([128, HPC, D], FP8, tag="wpl", name="wpl")

            def rope_ip(qk, tb, name, eng=None):
                """In-place RoPE on qk[:, tb-block] (holds the WS-scaled raw
                projection).  cos/sin are host-scaled by 1/WS, so the result
                is the true-scale rotated q/k.  The half-rotation runs on the
                PE as a signed permutation matmul (own psum pool so it never
                steals a qkv/score bank).  `eng` picks the engine for the two
                elementwise muls: gpsimd for latency-tolerant in-loop ropes,
                vector for phase-boundary ones."""
                t0, t1_ = TBE * tb, TBE * (tb + 1)
                blk = qk[:, t0:t1_]
                # pre-sin form: rot(blk*sin) == rot(blk)*sin because RoPE's
                # emb duplicates the frequencies in both halves (sin[d] ==
                # sin[(d+64)%128]).  This leaves only one DVE op (the psum
                # add) after the PE rotation instead of two.
                if eng is None:
                    eng = nc.vector
                m2, r1 = rope_mul(qk, tb, name, eng)
                rope_fin(qk, tb, name, m2, r1)

            def rope_mul(qk, tb, name, eng=None):
                """The two elementwise rope muls — only depend on the parked
                q/k block, so they can issue right after the park while the
                rot+add defer to the next schedule slot."""
                t0, t1_ = TBE * tb, TBE * (tb + 1)
                blk = qk[:, t0:t1_]
                if eng is None:
                    eng = nc.vector
                m2 = tmpp.tile([128, TBE], BF16, tag="r2", name=f"m2_{name}")
                eng.tensor_mul(m2[:], blk, sin_sb[:, t0:t1_])
                r1 = tmpp.tile([128, TBE], BF16, tag="r1", name=f"r1_{name}")
                eng.tensor_mul(r1[:], blk, cos_sb[:, t0:t1_])
                return m2, r1

            def rope_fin(qk, tb, name, m2, r1):
                t0, t1_ = TBE * tb, TBE * (tb + 1)
                blk = qk[:, t0:t1_]
                rps = ps_rp.tile([128, TBE], F32, tag="rp", name=f"rot_{name}")
                nc.tensor.matmul(rps[:], rot_sb[:], m2[:], start=True, stop=True)
                nc.vector.tensor_add(blk, r1[:], rps[:])

            def emit_proj(tt, on_act=False, tail=False):
                """y[128*tt:128*(tt+1), :] = sum_h oT_h[:, tt].T @ wp_h, as
                3-term fp8 DoubleRow over head pairs.  Psum carries WS*y;
                the drains (split across Act/DVE) fold in 1/WS.  One DMA per
                row-block normally; per-512-chunk DMAs on the tail so the
                last transfer isn't a serial 1MB copy."""
                yst = ysp.tile([128, D], BF16, tag="yst", name=f"yst{tt}")
                for db in range(D // 512):
                    ps = ps_mm.tile([128, 512], F32, tag="mm", name=f"psy{tt}_{db}")
                    idx = 0
                    for lhs_t, rhs_t in ((oTh_sb, wph_sb), (oTl_sb, wph_sb), (oTh_sb, wpl_sb)):
                        for g in range(HPC // 2):
                            nc.tensor.matmul(
                                ps[:],
                                lhs_t[:, 2 * g:2 * g + 2, 128 * tt:128 * (tt + 1)],
                                rhs_t[:, 2 * g:2 * g + 2, 512 * db:512 * (db + 1)],
                                start=(idx == 0), stop=(idx == 3 * (HPC // 2) - 1),
                                perf_mode=DR)
                            idx += 1
                    ys = yst[:, 512 * db:512 * (db + 1)]
                    if db % 2 == (0 if on_act else 1):
                        nc.scalar.mul(ys, ps[:], 1.0 / WS)
                    else:
                        nc.vector.tensor_scalar_mul(ys, ps[:], 1.0 / WS)
                    if tail:
                        nc.sync.dma_start(
                            y[128 * tt:128 * (tt + 1), 512 * db:512 * (db + 1)], ys)
                if not tail:
                    nc.sync.dma_start(y[128 * tt:128 * (tt + 1), :], yst[:])

            pair_bufs = {}
            pend_ropes = []

            def qkv_pair_steps(p2):
                """Generator: pair-p2 weight/x DMAs + fp8 qkv projection;
                yields once per t-block."""
                h = 2 * p2
                q_sbs = [None, None]
                k_sbs = [None, None]
                vext = [None, None]
                # fp8 weight tiles, [128, ND, 128]-shaped for DoubleRow pairs
                wq = [[wqkp.tile([128, ND, 128], FP8, tag=f"wq{i}{lv}", name=f"wq{i}{lv}_{p2}")
                       for lv in range(2)] for i in range(2)]
                wk = [[wqkp.tile([128, ND, 128], FP8, tag=f"wk{i}{lv}", name=f"wk{i}{lv}_{p2}")
                       for lv in range(2)] for i in range(2)]
                wv = [wvp.tile([128, ND, 256], FP8, tag=f"wv{lv}", name=f"wv{lv}_{p2}")
                      for lv in range(2)]
                vext[0] = vep.tile([128, NT, 129], BF16, tag="ve0", name=f"ve0_{p2}")
                vext[1] = vep.tile([128, NT, 129], BF16, tag="ve1", name=f"ve1_{p2}")
                # only the ones-columns need init (v-parks overwrite the rest);
                # 64.0 folds the WS weight scale out of the normalizer.
                nc.vector.memset(vext[0][:, :, 128:129], WS)
                nc.vector.memset(vext[1][:, :, 128:129], WS)
                q_sbs[0] = qkpool.tile([DH, t], BF16, tag="q0", name=f"q0_{p2}")
                k_sbs[0] = qkpool.tile([DH, t], BF16, tag="k0", name=f"k0_{p2}")
                q_sbs[1] = qkpool.tile([DH, t], BF16, tag="q1", name=f"q1_{p2}")
                k_sbs[1] = qkpool.tile([DH, t], BF16, tag="k1", name=f"k1_{p2}")
                pair_bufs[p2] = {"q": q_sbs, "k": k_sbs, "ve": vext}

                # x tiles for this pair (double-buffered across t-blocks)
                xts = {}
                loop_ropes = []

                def dma_x(tb):
                    xh_t = xtp.tile([128, ND, TBE], FP8, tag="xh", name=f"xh{p2}_{tb}")
                    xl_t = xtp.tile([128, ND, TBE], FP8, tag="xl", name=f"xl{p2}_{tb}")
                    t0, t1_ = TBE * tb, TBE * (tb + 1)
                    hd = ND // 2
                    for hf in range(2):
                        nc.sync.dma_start(
                            xh_t[:, hd * hf:hd * (hf + 1), :],
                            xTh[1024 * hf:1024 * (hf + 1), t0:t1_].rearrange(
                                "(j p) c -> p j c", p=128))
                    for hf in range(2):
                        nc.sync.dma_start(
                            xl_t[:, hd * hf:hd * (hf + 1), :],
                            xTl[1024 * hf:1024 * (hf + 1), t0:t1_].rearrange(
                                "(j p) c -> p j c", p=128))
                    xts[tb] = (xh_t, xl_t)

                # DMA order tuned so the first q0 matmul starts ~3us in and
                # every later consumer arrives just ahead of its first use:
                # wq0h, x-hi, wk0h, wq1h/wk1h, x-lo, lo-weights, wv.
                w0 = D * h
                w1 = D * (h + 1)
                xh_t = xtp.tile([128, ND, TBE], FP8, tag="xh", name=f"xh{p2}_0")
                xl_t = xtp.tile([128, ND, TBE], FP8, tag="xl", name=f"xl{p2}_0")
                hd = ND // 2
                qd = ND // 4
                nc.sync.dma_start(wq[0][0][:], wqh_hi[:, w0:w1])
                nc.sync.dma_start(
                    xh_t[:, 0:qd, :],
                    xTh[0:512, 0:TBE].rearrange("(j p) c -> p j c", p=128))
                nc.sync.dma_start(
                    xh_t[:, qd:2 * qd, :],
                    xTh[512:1024, 0:TBE].rearrange("(j p) c -> p j c", p=128))
                nc.sync.dma_start(wk[0][0][:], wkh_hi[:, w0:w1])
                nc.sync.dma_start(
                    xh_t[:, 2 * qd:3 * qd, :],
                    xTh[1024:1536, 0:TBE].rearrange("(j p) c -> p j c", p=128))
                nc.sync.dma_start(
                    xh_t[:, 3 * qd:ND, :],
                    xTh[1536:2048, 0:TBE].rearrange("(j p) c -> p j c", p=128))
                nc.sync.dma_start(wq[1][0][:], wqh_hi[:, w1:w1 + D])
                nc.sync.dma_start(wk[1][0][:], wkh_hi[:, w1:w1 + D])
                # term-1 lo weights arrive before term-2's x-lo
                nc.sync.dma_start(wq[0][1][:], wqh_lo[:, w0:w1])
                nc.sync.dma_start(wk[0][1][:], wkh_lo[:, w0:w1])
                nc.sync.dma_start(wq[1][1][:], wqh_lo[:, w1:w1 + D])
                nc.sync.dma_start(wk[1][1][:], wkh_lo[:, w1:w1 + D])
                for hf in range(2):
                    nc.sync.dma_start(
                        xl_t[:, hd * hf:hd * (hf + 1), :],
                        xTl[1024 * hf:1024 * (hf + 1), 0:TBE].rearrange(
                            "(j p) c -> p j c", p=128))
                xts[0] = (xh_t, xl_t)
                if p2 == 0:
                    # only tb0's cos/sin chunk up front; later chunks stream
                    # per-t-block so the x prefetches aren't queued behind
                    # 1MB of consts on the serial DMA device
                    nc.sync.dma_start(cos_sb[:, 0:TBE], cosT[:, 0:TBE])
                    nc.sync.dma_start(sin_sb[:, 0:TBE], sinTm[:, 0:TBE])
                nc.sync.dma_start(wv[0][:], wvh_hi[:, 2 * D * p2:2 * D * (p2 + 1)])
                nc.sync.dma_start(wv[1][:], wvh_lo[:, 2 * D * p2:2 * D * (p2 + 1)])
                if p2 == 0:
                    nc.sync.dma_start(rot_sb[:], rotm[:])
                    nc.sync.dma_start(bm_sb[:], bmask[:])
                    nc.sync.dma_start(id_sb[:], ident[:])
                else:
                    # prefetch the projection weights during pair-1 qkv
                    for hh in range(HPC):
                        nc.sync.dma_start(
                            wph_sb[:, hh, :], wpT_hi[128 * hh:128 * (hh + 1), :])
                        nc.sync.dma_start(
                            wpl_sb[:, hh, :], wpT_lo[128 * hh:128 * (hh + 1), :])

                for tb in range(NTB):
                    t0, t1_ = TBE * tb, TBE * (tb + 1)
                    for qk, tb_, nm_, m2_, r1_ in loop_ropes:
                        rope_fin(qk, tb_, nm_, m2_, r1_)
                    loop_ropes.clear()
                    if tb + 1 < NTB:
                        dma_x(tb + 1)
                        if p2 == 0:
                            nt0, nt1 = TBE * (tb + 1), TBE * (tb + 2)
                            nc.sync.dma_start(cos_sb[:, nt0:nt1], cosT[:, nt0:nt1])
                            nc.sync.dma_start(sin_sb[:, nt0:nt1], sinTm[:, nt0:nt1])
                    xh_t, xl_t = xts.pop(tb)
                    ps_q0 = ps_mm.tile([128, TBE], F32, tag="mm", name=f"psq0_{p2}_{tb}")
                    ps_k0 = ps_mm.tile([128, TBE], F32, tag="mm", name=f"psk0_{p2}_{tb}")
                    ps_q1 = ps_mm.tile([128, TBE], F32, tag="mm", name=f"psq1_{p2}_{tb}")
                    ps_k1 = ps_mm.tile([128, TBE], F32, tag="mm", name=f"psk1_{p2}_{tb}")
                    nvp = (TTPB + 1) // 2
                    ps_vs = [
                        ps_sm.tile([128, 512], F32, tag="sm", name=f"psv{p2}_{tb}_{i}")
                        for i in range(nvp)
                    ]
                    # 3-term fp8 accumulation: wh*xh + wl*xh + wh*xl,
                    # term-major (matches the tb-0 DMA arrival order); each
                    # psum parks right after its last matmul so the bank
                    # frees and the copy overlaps the remaining matmuls.
                    last_tb = tb == NTB - 1
                    qk_psums = ((ps_q0, wq[0], q_sbs[0]), (ps_k0, wk[0], k_sbs[0]),
                                (ps_q1, wq[1], q_sbs[1]), (ps_k1, wk[1], k_sbs[1]))

                    def qk_sweep():
                        for term in range(3):
                            wlv = 1 if term == 1 else 0
                            xt = xl_t if term == 2 else xh_t
                            first = term == 0
                            last = term == 2
                            for i_, (ps, wt, dst) in enumerate(qk_psums):
                                for j in range(NJ):
                                    js = slice(2 * j, 2 * j + 2)
                                    nc.tensor.matmul(ps[:], wt[wlv][:, js, :], xt[:, js, :],
                                                     start=(first and j == 0),
                                                     stop=(last and j == NJ - 1),
                                                     perf_mode=DR)
                                if last:
                                    # split parks across Act/DVE once
                                    # attention exps share the Act queue
                                    if tb > 0 and i_ % 2 == 1:
                                        nc.vector.tensor_copy(dst[:, t0:t1_], ps[:])
                                    else:
                                        nc.scalar.copy(dst[:, t0:t1_], ps[:])

                    def v_sweep():
                        for term in range(3):
                            wvt = wv[1 if term == 1 else 0]
                            xt = xl_t if term == 2 else xh_t
                            first = term == 0
                            last = term == 2
                            for j in range(NJ):
                                js = slice(2 * j, 2 * j + 2)
                                for tt in range(TTPB):
                                    nc.tensor.matmul(
                                        ps_vs[tt // 2][:, 256 * (tt % 2):256 * (tt % 2) + 256],
                                        xt[:, js, 128 * tt:128 * (tt + 1)],
                                        wvt[:, js, :],
                                        start=(first and j == 0 and tt % 2 == 0),
                                        stop=(last and j == NJ - 1),
                                        skip_group_check=True, perf_mode=DR)
                        for tt in range(TTPB):
                            gt = tb * TTPB + tt
                            o0 = 256 * (tt % 2)
                            vc = nc.vector.tensor_copy if (tb > 0 and tt % 2) else nc.scalar.copy
                            vc(vext[0][:, gt, 0:128], ps_vs[tt // 2][:, o0:o0 + 128])
                            vc(vext[1][:, gt, 0:128], ps_vs[tt // 2][:, o0 + 128:o0 + 256])

                    # last t-block: V first so its psums drain under the q/k
                    # matmuls and the pair boundary ends on parks+ropes only
                    if last_tb and tb > 0:
                        v_sweep()
                        qk_sweep()
                    else:
                        qk_sweep()
                        v_sweep()
                    # RoPE policy: in-loop ropes are queued and ISSUED at the
                    # start of the next t-block slot, so their rot matmuls
                    # land ahead of the 15us qkv stream in the in-order PE
                    # queue (a rope add stuck behind a whole t-block blocks
                    # every later DVE op).  Pair 0 rotates all four heads
                    # in-loop; pair 1 only its h2 head (h3's rotate in phase
                    # 3 via scheduler-placed consume_ropes).  The last
                    # t-block always defers.
                    if tb != NTB - 1:
                        heads = [0, 1] if p2 == 1 else [0]
                        for par in heads:
                            for qk, pfx in ((q_sbs[par], "q"), (k_sbs[par], "k")):
                                nm = f"{pfx}{par}_{p2}_{tb}"
                                # pair-0 muls on idle GpSimd; pair-1's on DVE
                                # (phase-2 Pool is busy with o-splits)
                                meng = nc.gpsimd if p2 == 0 else nc.vector
                                m2_, r1_ = rope_mul(qk, tb, nm, eng=meng)
                                loop_ropes.append((qk, tb, nm, m2_, r1_))
                        if p2 == 0:
                            pend_ropes.append((q_sbs[1], tb, f"q1_{p2}_{tb}"))
                            pend_ropes.append((k_sbs[1], tb, f"k1_{p2}_{tb}"))
                    else:
                        # the pair's own (h-even) tail ropes go to the FRONT
                        # so schedule-placed consume_ropes() can rotate them
                        # before the partner-head ones
                        pend_ropes.insert(0, (k_sbs[0], tb, f"k0_{p2}_{tb}"))
                        pend_ropes.insert(0, (q_sbs[0], tb, f"q0_{p2}_{tb}"))
                        pend_ropes.append((q_sbs[1], tb, f"q1_{p2}_{tb}"))
                        pend_ropes.append((k_sbs[1], tb, f"k1_{p2}_{tb}"))
                    yield

            def attention_steps(h, quotas, pipelined, do_proj=False):
                """Generator for head h's attention, yielding once per chunk."""
                par = h % 2
                bufs = pair_bufs[h // 2]
                q_sb, k_sb = bufs["q"][par], bufs["k"][par]
                ve = bufs["ve"][par]
                if par == 0 and pend_ropes:
                    mine = [e for e in pend_ropes if e[0] is q_sb or e[0] is k_sb]
                    rest = [e for e in pend_ropes if not (e[0] is q_sb or e[0] is k_sb)]
                    pend_ropes[:] = mine + rest

                def stage_a(ib):
                    """scores + exp + diagonal mask for i-block ib; the score
                    matmul and exp narrow to the causal range on diagonal
                    j-tiles."""
                    i0 = sb * ib
                    jt_max = (i0 + sb) // 128 - 1  # inclusive
                    pts = [None] * (jt_max + 1)
                    for jt in range(jt_max + 1):
                        m = jt - NIC * ib
                        off = 128 * m if m > 0 else 0
                        s_ps = ps_mm.tile([128, sb], F32, tag="mm", name=f"s{h}_{ib}_{jt}")
                        nc.tensor.matmul(
                            s_ps[:, off:sb],
                            k_sb[:, 128 * jt:128 * (jt + 1)],
                            q_sb[:, i0 + off:i0 + sb],
                            start=True, stop=True)
                        pt_t = ptp.tile([128, sb], BF16, tag="pt", name=f"pt{h}_{ib}_{jt}")
                        nc.scalar.activation(pt_t[:, off:sb], s_ps[:, off:sb], AF.Exp, scale=SCALE)
                        if m >= 0:
                            pm = pt_t[:, 128 * m:128 * (m + 1)]
                            nc.vector.tensor_mul(pm, pm, bm_sb[:])
                        pts[jt] = pt_t
                    return pts

                def stage_b(ib, pts, bi):
                    """PV + normalize + transpose + fp8 hi/lo split."""
                    i0 = sb * ib

                    def finish(ic, pv):
                        rc = smallp.tile([128, 1], F32, tag="rc", name=f"rc{h}_{ib}_{ic}")
                        nc.vector.reciprocal(rc[:], pv[:, 128:129])
                        o_sb = smallp.tile([128, 128], BF16, tag="o", name=f"o{h}_{ib}_{ic}")
                        nc.vector.tensor_scalar_mul(o_sb[:], pv[:, 0:128], rc[:])
                        ot_ps = ps_mm.tile([128, 128], BF16, tag="mm", name=f"otp{h}_{ib}_{ic}")
                        nc.tensor.transpose(ot_ps[:], o_sb[:], id_sb[:])
                        c0 = i0 + 128 * ic
                        hs = oTh_sb[:, h, c0:c0 + 128]
                        if do_proj:
                            # proj-critical head: split straight off the psum
                            # on Act + DVE (no Pool latency in the chain)
                            nc.scalar.copy(hs, ot_ps[:])
                            nc.vector.scalar_tensor_tensor(
                                oTl_sb[:, h, c0:c0 + 128], hs, -1.0, ot_ps[:],
                                mybir.AluOpType.mult, mybir.AluOpType.add)
                        else:
                            otb = smallp.tile([128, 128], BF16, tag="otb", name=f"otb{h}_{ib}_{ic}")
                            nc.vector.tensor_copy(otb[:], ot_ps[:])
                            # fp8 hi/lo split on GpSimd (SBUF-only engine)
                            nc.gpsimd.tensor_copy(hs, otb[:])
                            nc.gpsimd.tensor_sub(oTl_sb[:, h, c0:c0 + 128], otb[:], hs)

                    prev = None
                    for ic in range(NIC):
                        last_jt = NIC * ib + ic
                        pv = ps_sm.tile([128, 129], F32, tag="sm", name=f"pv{h}_{ib}_{ic}")
                        for jt in range(last_jt + 1):
                            nc.tensor.matmul(
                                pv[:],
                                pts[jt][:, 128 * ic:128 * (ic + 1)],
                                ve[:, jt, :],
                                start=(jt == 0), stop=(jt == last_jt))
                        if prev is not None:
                            finish(*prev)
                            if do_proj:
                                emit_proj(NIC * ib + prev[0], on_act=False,
                                          tail=(ib == NSB - 1))
                        prev = (ic, pv)
                    finish(*prev)
                    if do_proj:
                        emit_proj(NIC * ib + prev[0],
                                  on_act=(ib == NSB - 1), tail=(ib == NSB - 1))
                    consume_ropes(quotas[bi])

                if pipelined:
                    pts_prev = None
                    for ib in range(NSB):
                        pts_cur = stage_a(ib)
                        yield
                        if pts_prev is not None:
                            stage_b(ib - 1, pts_prev, ib - 1)
                            yield
                        pts_prev = pts_cur
                    stage_b(NSB - 1, pts_prev, NSB - 1)
                    yield
                else:
                    for ib in range(NSB):
                        pts = stage_a(ib)
                        yield
                        stage_b(ib, pts, ib)
                        yield

            def stepn(g, n):
                for _ in range(n):
                    next(g)

            def run(g):
                for _ in g:
                    pass

            def consume_ropes(n):
                for qk, tb_, nm_ in pend_ropes[:n]:
                    rope_ip(qk, tb_, nm_)
                del pend_ropes[:n]

            # ---- schedule -------------------------------------------
            # pair 0 qkv alone; h0/h1 attention chunks (mutually interleaved
            # so one head's scores fill the other's exp latency) interleaved
            # with pair 1's qkv t-blocks; h2/h3 likewise interleaved with
            # each other and the output projection folded into h3's B chunks.
            if NTB >= 4 and NSB >= 4:
                Z = [0, 0, 0, 0]
                # h0's attention starts one i-block behind pair-0's qkv
                # t-blocks (its k/v prefix is complete by then), filling
                # pair-0's otherwise idle Act with exps; deferred ropes are
                # consumed at explicit schedule points, each before any chunk
                # that reads the rotated tile.
                q0 = qkv_pair_steps(0)
                a0 = attention_steps(0, Z, True)
                stepn(q0, 2)        # tb0 tb1 (tb0's ropes issue at tb1 start)
                stepn(a0, 1)        # A0
                stepn(q0, 1)        # tb2 (ropes tb1)
                stepn(a0, 2)        # A1 B0
                stepn(q0, 1)        # tb3 (ropes tb2)
                consume_ropes(2)    # q0/k0 pair0-tb3
                stepn(a0, 3)        # A2 B1 A3
                consume_ropes(4)    # q1/k1 pair0-tb0, tb1
                a1 = attention_steps(1, Z, True)
                q1 = qkv_pair_steps(1)
                a2 = attention_steps(2, Z, True)
                stepn(a0, 1)        # B2
                stepn(a1, 1)        # A0
                stepn(q1, 1)        # pair1 tb0
                consume_ropes(2)    # q1/k1 pair0-tb2
                stepn(a1, 2)        # A1 B0
                stepn(a0, 1)        # B3
                stepn(q1, 1)        # tb1 (ropes p1-tb0)
                consume_ropes(2)    # q1/k1 pair0-tb3
                stepn(a2, 1)        # h2.A0
                stepn(a1, 2)        # A2 B1
                stepn(q1, 1)        # tb2 (ropes p1-tb1)
                stepn(a2, 2)        # h2.A1 B0
                stepn(a1, 2)        # A3 B2
                stepn(q1, 1)        # tb3 (ropes p1-tb2)
                consume_ropes(2)    # q0/k0 pair1-tb3
                stepn(a2, 2)        # h2.A2 B1
                run(a1)             # B3
                run(a0)
                run(q0)
                run(q1)
                # h3's first chunks (and ib0's projection) pull into the
                # phase-2 tail where Act still has headroom; phase 3 is the
                # remainder with h2's tail interleaved.
                a3 = attention_steps(3, Z, True, do_proj=True)
                consume_ropes(2)    # q1/k1 pair1-tb3
                stepn(a3, 3)        # A0 A1 B0 (+proj ib0)
                stepn(a2, 1)        # h2.A3
                stepn(a2, 1)        # h2.B2
                stepn(a3, 2)        # A2 B1 (+proj ib1)
                stepn(a2, 1)        # h2.B3
                stepn(a3, 1)        # A3
                run(a3)             # B2 B3 (+proj ib2, ib3)
                run(a2)
            else:
                run(qkv_pair_steps(0))
                run(attention_steps(0, [2, 2, 0, 0], True))
                run(attention_steps(1, [0, 0, 0, 0], True))
                run(qkv_pair_steps(1))
                run(attention_steps(2, [2, 2, 0, 0], True))
                run(attention_steps(3, [0, 0, 0, 0], True, do_proj=True))

    nc.compile()
    return nc


def host_consts(t=T):
    """RoPE cos/sin (scaled by 1/WS to fold out the fp8 weight pre-scale),
    causal big-mask, identity, signed half-rotation."""
    inv = (1.0 / (np.float32(10000.0) ** (np.arange(0, DH, 2, dtype=np.float32) / np.float32(DH)))).astype(np.float32)
    tt = np.arange(t, dtype=np.float32)
    fr = np.outer(tt, inv).astype(np.float32)       # [t, 64]
    emb = np.concatenate([fr, fr], axis=1)          # [t, 128]
    cosT = np.ascontiguousarray(np.cos(emb).T.astype(np.float32)) / np.float32(WS)
    sinTm = np.ascontiguousarray(np.sin(emb).T.astype(np.float32)) / np.float32(WS)
    jj = np.arange(128)[:, None]
    cc = np.arange(128)[None, :]
    bmask = (cc >= jj).astype(np.float32)
    ident = np.eye(128, dtype=np.float32)
    # signed half-rotation: (rotm.T @ x)[d] = -x[d+64] for d<64, x[d-64] else
    rotm = np.zeros((128, 128), dtype=np.float32)
    for d in range(64):
        rotm[d + 64, d] = -1.0
        rotm[d, d + 64] = 1.0
    return cosT, sinTm, bmask, ident, rotm


def _warrange(w):
    """[128*nh rows, D] head-major weight slice -> [128, nh*D] sbuf-ready layout:
    block h, col di*128+c of partition p  =  w[128*h + c, 128*di + p]."""
    nh = w.shape[0] // 128
    d = w.shape[1]
    out = np.empty((128, nh * d), dtype=w.dtype)
    for h in range(nh):
        a = w[128 * h:128 * (h + 1), :].T.reshape(d // 128, 128, 128)  # [di, p, c]
        out[:, d * h:d * (h + 1)] = a.transpose(1, 0, 2).reshape(128, d)
    return out


def _wvarrange(w):
    """[512 rows, D] 4-head v-weights -> [128, 2*2*D]: per pair, di-major blocks of
    [even-head 128 cols | odd-head 128 cols]."""
    d = w.shape[1]
    blocks = []
    for p2 in range(2):
        e = w[256 * p2:256 * p2 + 128, :].T.reshape(d // 128, 128, 128)
        o = w[256 * p2 + 128:256 * p2 + 256, :].T.reshape(d // 128, 128, 128)
        pair = np.concatenate([e, o], axis=2)          # [di, p, 256]
        blocks.append(pair.transpose(1, 0, 2).reshape(128, 2 * d))
    return np.concatenate(blocks, axis=1)


FP8NP = ml_dtypes.float8_e4m3


def _split8(a):
    """Error-compensated e4m3 split: a ~= hi + lo (fp32 in, fp8 out pair)."""
    a = np.asarray(a, dtype=np.float32)
    hi = a.astype(FP8NP)
    lo = (a - hi.astype(np.float32)).astype(FP8NP)
    return hi, lo


def shard_inputs(x, w_qkv, w_proj, t=T, pv_dt="bfloat16"):
    """Build the 8 per-core input maps (host does the fp8 splits)."""
    bdt = ml_dtypes.bfloat16
    cosT, sinTm, bmask, ident, rotm = host_consts(t)
    cosT = cosT.astype(bdt)
    sinTm = sinTm.astype(bdt)
    bmask = bmask.astype(bdt)
    ident = ident.astype(bdt)
    rotm = rotm.astype(bdt)
    d = x.shape[2]
    ws = np.float32(WS)
    in_maps = []
    xs = {}
    for b in range(x.shape[0]):
        xs[b] = _split8(np.ascontiguousarray(x[b].T))
    for c in range(8):
        b, g = divmod(c, 4)
        s0, s1 = 512 * g, 512 * (g + 1)
        wq_hi, wq_lo = _split8(w_qkv[s0:s1, :] * ws)
        wk_hi, wk_lo = _split8(w_qkv[d + s0:d + s1, :] * ws)
        wv_hi, wv_lo = _split8(w_qkv[2 * d + s0:2 * d + s1, :] * ws)
        wp_hi, wp_lo = _split8(np.ascontiguousarray(w_proj[:, s0:s1].T) * ws)
        in_maps.append(dict(
            xTh=xs[b][0], xTl=xs[b][1],
            wqh_hi=_warrange(wq_hi), wqh_lo=_warrange(wq_lo),
            wkh_hi=_warrange(wk_hi), wkh_lo=_warrange(wk_lo),
            wvh_hi=_wvarrange(wv_hi), wvh_lo=_wvarrange(wv_lo),
            wpT_hi=wp_hi, wpT_lo=wp_lo,
            cosT=cosT, sinTm=sinTm, bmask=bmask, ident=ident, rotm=rotm,
        ))
    return in_maps


_NC_CACHE = {}


def get_nc(t=T, mm_dt="float32r", pv_dt="bfloat16"):
    key = (t, mm_dt, pv_dt)
    if key not in _NC_CACHE:
        _NC_CACHE[key] = build_nc(t=t, mm_dt=mm_dt, pv_dt=pv_dt)
    return _NC_CACHE[key]


def kernel(x, w_qkv, w_proj):
    x = np.asarray(x, dtype=np.float32)
    w_qkv = np.asarray(w_qkv, dtype=np.float32)
    w_proj = np.asarray(w_proj, dtype=np.float32)
    b_, t_, d_ = x.shape
    in_maps = shard_inputs(x, w_qkv, w_proj, t=t_)
    nc = get_nc(t=t_)
    res = run_bass_kernel_spmd(nc, in_maps, list(range(8))).results
    out = np.zeros((b_, t_, d_), dtype=np.float32)
    for c in range(8):
        b, _ = divmod(c, 4)
        out[b] += res[c]["y"]
    return out


# revision 72
# speedup vs baseline: 1.0749x; 1.0017x over previous
"""Trainium2 Bass kernel: causal self-attention with RoPE (B=2, T=2048, D=2048, H=16).

Sharding: 8 cores = 2-way data parallel over batch x 4-way tensor parallel over
heads.  Core c = 4*b + g computes batch b, heads 4g..4g+3, and produces a
partial output y_partial = attn_out[:, heads_g] @ w_proj[:, heads_g].T which the
host sums over g.

Per-core pipeline (v3 — fp8 DoubleRow for the projections):
  - qkv projection runs in fp8e4 with MatmulPerfMode.DoubleRow (2 K-planes per
    instruction at 0.5 cycles/row) using a host-side error-compensated split:
    w = wh + wl, x = xh + xl (each term an e4m3 tensor), accumulating
    wh*xh + wl*xh + wh*xl into the same psum (12N cycles per 128x2048-contract
    tile vs 16N in fp32r/bf16).  Weights are pre-scaled by 2^6 on the host so
    the hi/lo parts stay out of e4m3's subnormal floor; the 2^-6 is folded
    into the (host-precomputed) rope cos/sin for q/k, into the ones-columns of
    the V staging tiles (memset 64.0: den = 64*sum(p), o = num*(1/den)*64
    cancels), and into the final y drain (scaled copy) for the projection.
  - scores/PV stay bf16; score matmuls + exp narrow to the causal range on the
    diagonal j-tiles (the skipped top-left pt region is never read).
  - attention output oT is split to fp8 hi/lo on the (otherwise idle) GpSimd
    engine; the output projection is again 3-term DoubleRow over head pairs.
  - issue-level interleave: pair-1 qkv t-blocks are issued between h0/h1
    attention chunks so qkv matmuls fill the exp-latency PE gaps.
"""

import sys

import numpy as np
import ml_dtypes

for _p in ("/opt/trn_rl_repo", "/root/.axon_site/_ro/trn_rl_repo"):
    if _p not in sys.path:
        sys.path.append(_p)

import concourse.bass as bass
import concourse.bacc as bacc
import concourse.tile as tile
from concourse import mybir
from concourse.bass_utils import run_bass_kernel_spmd

F32 = mybir.dt.float32
BF16 = mybir.dt.bfloat16
FP8 = mybir.dt.float8e4
AF = mybir.ActivationFunctionType
DR = mybir.MatmulPerfMode.DoubleRow

B, T, D, H = 2, 2048, 2048, 16
HPC = H // 4  # heads per core (4-way head TP)
DH = D // H   # 128
SCALE = float(DH) ** -0.5
WS = 64.0     # host-side weight pre-scale (2^6)

TB = 512      # qkv-projection t-block (psum free width)
SB = 512      # attention i-block (score free width)


def build_nc(t=T, mm_dt="float32r", pv_dt="bfloat16"):
    """Build the SPMD per-core program.  `t` is the sequence length (smaller
    values are used for simulator validation).  mm_dt/pv_dt kept for test.py
    compatibility (ignored: matmuls are fp8-DoubleRow / bf16)."""
    NT = t // 128    # token tiles
    TBE = min(TB, t)
    NTB = t // TBE   # qkv t-blocks
    sb = min(SB, t)
    NSB = t // sb    # attention i-blocks
    NIC = sb // 128  # i-chunks per i-block
    ND = D // 128    # contraction d-tiles
    NJ = ND // 2     # DoubleRow K-pair count for the qkv contraction
    TTPB = TBE // 128

    nc = bacc.Bacc("TRN2", target_bir_lowering=False, debug=False)

    xTh = nc.dram_tensor("xTh", [D, t], FP8, kind="ExternalInput").ap()
    xTl = nc.dram_tensor("xTl", [D, t], FP8, kind="ExternalInput").ap()
    wqh_hi = nc.dram_tensor("wqh_hi", [128, HPC * D], FP8, kind="ExternalInput").ap()
    wqh_lo = nc.dram_tensor("wqh_lo", [128, HPC * D], FP8, kind="ExternalInput").ap()
    wkh_hi = nc.dram_tensor("wkh_hi", [128, HPC * D], FP8, kind="ExternalInput").ap()
    wkh_lo = nc.dram_tensor("wkh_lo", [128, HPC * D], FP8, kind="ExternalInput").ap()
    wvh_hi = nc.dram_tensor("wvh_hi", [128, (HPC // 2) * 2 * D], FP8, kind="ExternalInput").ap()
    wvh_lo = nc.dram_tensor("wvh_lo", [128, (HPC // 2) * 2 * D], FP8, kind="ExternalInput").ap()
    wpT_hi = nc.dram_tensor("wpT_hi", [HPC * DH, D], FP8, kind="ExternalInput").ap()
    wpT_lo = nc.dram_tensor("wpT_lo", [HPC * DH, D], FP8, kind="ExternalInput").ap()
    cosT = nc.dram_tensor("cosT", [DH, t], BF16, kind="ExternalInput").ap()
    sinTm = nc.dram_tensor("sinTm", [DH, t], BF16, kind="ExternalInput").ap()
    bmask = nc.dram_tensor("bmask", [128, 128], BF16, kind="ExternalInput").ap()
    ident = nc.dram_tensor("ident", [128, 128], BF16, kind="ExternalInput").ap()
    rotm = nc.dram_tensor("rotm", [128, 128], BF16, kind="ExternalInput").ap()
    y = nc.dram_tensor("y", [t, D], BF16, kind="ExternalOutput").ap()

    with tile.TileContext(nc) as tc:
        with (
            tc.tile_pool(name="consts", bufs=1) as cpool,
            tc.tile_pool(name="oTp", bufs=1) as opool,
            tc.tile_pool(name="qkp", bufs=2) as qkpool,
            tc.tile_pool(name="xtp", bufs=2) as xtp,
            tc.tile_pool(name="wqkp", bufs=1) as wqkp,
            tc.tile_pool(name="wvp", bufs=1) as wvp,
            tc.tile_pool(name="wpj", bufs=1) as wpj,
            tc.tile_pool(name="vep", bufs=2) as vep,
            tc.tile_pool(name="ptp", bufs=max(2 * NT + 1, NT + 1)) as ptp,
            tc.tile_pool(name="tmpp", bufs=6) as tmpp,
            tc.tile_pool(name="smallp", bufs=8) as smallp,
            tc.tile_pool(name="ysp", bufs=3) as ysp,
            tc.tile_pool(name="ps_mm", bufs=5, space="PSUM") as ps_mm,
            tc.tile_pool(name="ps_sm", bufs=2, space="PSUM") as ps_sm,
            tc.tile_pool(name="ps_rp", bufs=1, space="PSUM") as ps_rp,
        ):
            # PE warmup: dummy matmuls on a memset tile bridge the initial
            # DMA wait so the p-state ramp (half-rate for 3us after idle)
            # completes before the first real matmul.
            if t >= 2048:
                wrm = cpool.tile([128, 512], BF16, tag="wrm", name="wrm")
                nc.vector.memset(wrm[:], 0.0)
                for wi in range(9):
                    wps = ps_rp.tile([128, 512], F32, tag="rp", name=f"warm{wi}")
                    nc.tensor.matmul(wps[:], wrm[:, 0:128], wrm[:],
                                     start=True, stop=True)

            cos_sb = cpool.tile([DH, t], BF16, tag="cos")
            sin_sb = cpool.tile([DH, t], BF16, tag="sin")
            bm_sb = cpool.tile([128, 128], BF16, tag="bm")
            id_sb = cpool.tile([128, 128], BF16, tag="id")
            rot_sb = cpool.tile([128, 128], BF16, tag="rot")
            # fp8 hi/lo attention outputs, head-plane layout for DoubleRow
            oTh_sb = opool.tile([128, HPC, t], FP8, tag="oTh", name="oTh")
            oTl_sb = opool.tile([128, HPC, t], FP8, tag="oTl", name="oTl")
            wph_sb = wpj.tile([128, HPC, D], FP8, tag="wph", name="wph")
            wpl_sb = wpj.tile([128, HPC, D], FP8, tag="wpl", name="wpl")

            def rope_ip(qk, tb, name, eng=None):
                """In-place RoPE on qk[:, tb-block] (holds the WS-scaled raw
                projection).  cos/sin are host-scaled by 1/WS, so the result
                is the true-scale rotated q/k.  The half-rotation runs on the
                PE as a signed permutation matmul (own psum pool so it never
                steals a qkv/score bank).  `eng` picks the engine for the two
                elementwise muls: gpsimd for latency-tolerant in-loop ropes,
                vector for phase-boundary ones."""
                t0, t1_ = TBE * tb, TBE * (tb + 1)
                blk = qk[:, t0:t1_]
                # pre-sin form: rot(blk*sin) == rot(blk)*sin because RoPE's
                # emb duplicates the frequencies in both halves (sin[d] ==
                # sin[(d+64)%128]).  This leaves only one DVE op (the psum
                # add) after the PE rotation instead of two.
                if eng is None:
                    eng = nc.vector
                m2, r1 = rope_mul(qk, tb, name, eng)
                rope_fin(qk, tb, name, m2, r1)

            def rope_mul(qk, tb, name, eng=None):
                """The two elementwise rope muls — only depend on the parked
                q/k block, so they can issue right after the park while the
                rot+add defer to the next schedule slot."""
                t0, t1_ = TBE * tb, TBE * (tb + 1)
                blk = qk[:, t0:t1_]
                if eng is None:
                    eng = nc.vector
                m2 = tmpp.tile([128, TBE], BF16, tag="r2", name=f"m2_{name}")
                eng.tensor_mul(m2[:], blk, sin_sb[:, t0:t1_])
                r1 = tmpp.tile([128, TBE], BF16, tag="r1", name=f"r1_{name}")
                eng.tensor_mul(r1[:], blk, cos_sb[:, t0:t1_])
                return m2, r1

            def rope_fin(qk, tb, name, m2, r1):
                t0, t1_ = TBE * tb, TBE * (tb + 1)
                blk = qk[:, t0:t1_]
                rps = ps_rp.tile([128, TBE], F32, tag="rp", name=f"rot_{name}")
                nc.tensor.matmul(rps[:], rot_sb[:], m2[:], start=True, stop=True)
                nc.vector.tensor_add(blk, r1[:], rps[:])

            def emit_proj(tt, on_act=False, tail=False):
                """y[128*tt:128*(tt+1), :] = sum_h oT_h[:, tt].T @ wp_h, as
                3-term fp8 DoubleRow over head pairs.  Psum carries WS*y;
                the drains (split across Act/DVE) fold in 1/WS.  One DMA per
                row-block normally; per-512-chunk DMAs on the tail so the
                last transfer isn't a serial 1MB copy."""
                yst = ysp.tile([128, D], BF16, tag="yst", name=f"yst{tt}")
                for db in range(D // 512):
                    ps = ps_mm.tile([128, 512], F32, tag="mm", name=f"psy{tt}_{db}")
                    idx = 0
                    for lhs_t, rhs_t in ((oTh_sb, wph_sb), (oTl_sb, wph_sb), (oTh_sb, wpl_sb)):
                        for g in range(HPC // 2):
                            nc.tensor.matmul(
                                ps[:],
                                lhs_t[:, 2 * g:2 * g + 2, 128 * tt:128 * (tt + 1)],
                                rhs_t[:, 2 * g:2 * g + 2, 512 * db:512 * (db + 1)],
                                start=(idx == 0), stop=(idx == 3 * (HPC // 2) - 1),
                                perf_mode=DR)
                            idx += 1
                    ys = yst[:, 512 * db:512 * (db + 1)]
                    if db % 2 == (0 if on_act else 1):
                        nc.scalar.mul(ys, ps[:], 1.0 / WS)
                    else:
                        nc.vector.tensor_scalar_mul(ys, ps[:], 1.0 / WS)
                    if tail:
                        nc.sync.dma_start(
                            y[128 * tt:128 * (tt + 1), 512 * db:512 * (db + 1)], ys)
                if not tail:
                    nc.sync.dma_start(y[128 * tt:128 * (tt + 1), :], yst[:])

            pair_bufs = {}
            pend_ropes = []

            def qkv_pair_steps(p2):
                """Generator: pair-p2 weight/x DMAs + fp8 qkv projection;
                yields once per t-block."""
                h = 2 * p2
                q_sbs = [None, None]
                k_sbs = [None, None]
                vext = [None, None]
                # fp8 weight tiles, [128, ND, 128]-shaped for DoubleRow pairs
                wq = [[wqkp.tile([128, ND, 128], FP8, tag=f"wq{i}{lv}", name=f"wq{i}{lv}_{p2}")
                       for lv in range(2)] for i in range(2)]
                wk = [[wqkp.tile([128, ND, 128], FP8, tag=f"wk{i}{lv}", name=f"wk{i}{lv}_{p2}")
                       for lv in range(2)] for i in range(2)]
                wv = [wvp.tile([128, ND, 256], FP8, tag=f"wv{lv}", name=f"wv{lv}_{p2}")
                      for lv in range(2)]
                vext[0] = vep.tile([128, NT, 129], BF16, tag="ve0", name=f"ve0_{p2}")
                vext[1] = vep.tile([128, NT, 129], BF16, tag="ve1", name=f"ve1_{p2}")
                # only the ones-columns need init (v-parks overwrite the rest);
                # 64.0 folds the WS weight scale out of the normalizer.
                nc.vector.memset(vext[0][:, :, 128:129], WS)
                nc.vector.memset(vext[1][:, :, 128:129], WS)
                q_sbs[0] = qkpool.tile([DH, t], BF16, tag="q0", name=f"q0_{p2}")
                k_sbs[0] = qkpool.tile([DH, t], BF16, tag="k0", name=f"k0_{p2}")
                q_sbs[1] = qkpool.tile([DH, t], BF16, tag="q1", name=f"q1_{p2}")
                k_sbs[1] = qkpool.tile([DH, t], BF16, tag="k1", name=f"k1_{p2}")
                pair_bufs[p2] = {"q": q_sbs, "k": k_sbs, "ve": vext}

                # x tiles for this pair (double-buffered across t-blocks)
                xts = {}
                loop_ropes = []

                def dma_x(tb):
                    xh_t = xtp.tile([128, ND, TBE], FP8, tag="xh", name=f"xh{p2}_{tb}")
                    xl_t = xtp.tile([128, ND, TBE], FP8, tag="xl", name=f"xl{p2}_{tb}")
                    t0, t1_ = TBE * tb, TBE * (tb + 1)
                    hd = ND // 2
                    for hf in range(2):
                        nc.sync.dma_start(
                            xh_t[:, hd * hf:hd * (hf + 1), :],
                            xTh[1024 * hf:1024 * (hf + 1), t0:t1_].rearrange(
                                "(j p) c -> p j c", p=128))
                    for hf in range(2):
                        nc.sync.dma_start(
                            xl_t[:, hd * hf:hd * (hf + 1), :],
                            xTl[1024 * hf:1024 * (hf + 1), t0:t1_].rearrange(
                                "(j p) c -> p j c", p=128))
                    xts[tb] = (xh_t, xl_t)

                # DMA order tuned so the first q0 matmul starts ~3us in and
                # every later consumer arrives just ahead of its first use:
                # wq0h, x-hi, wk0h, wq1h/wk1h, x-lo, lo-weights, wv.
                w0 = D * h
                w1 = D * (h + 1)
                xh_t = xtp.tile([128, ND, TBE], FP8, tag="xh", name=f"xh{p2}_0")
                xl_t = xtp.tile([128, ND, TBE], FP8, tag="xl", name=f"xl{p2}_0")
                hd = ND // 2
                qd = ND // 4
                nc.sync.dma_start(wq[0][0][:], wqh_hi[:, w0:w1])
                nc.sync.dma_start(
                    xh_t[:, 0:qd, :],
                    xTh[0:512, 0:TBE].rearrange("(j p) c -> p j c", p=128))
                nc.sync.dma_start(
                    xh_t[:, qd:2 * qd, :],
                    xTh[512:1024, 0:TBE].rearrange("(j p) c -> p j c", p=128))
                nc.sync.dma_start(wk[0][0][:], wkh_hi[:, w0:w1])
                nc.sync.dma_start(
                    xh_t[:, 2 * qd:3 * qd, :],
                    xTh[1024:1536, 0:TBE].rearrange("(j p) c -> p j c", p=128))
                nc.sync.dma_start(
                    xh_t[:, 3 * qd:ND, :],
                    xTh[1536:2048, 0:TBE].rearrange("(j p) c -> p j c", p=128))
                nc.sync.dma_start(wq[1][0][:], wqh_hi[:, w1:w1 + D])
                nc.sync.dma_start(wk[1][0][:], wkh_hi[:, w1:w1 + D])
                # term-1 lo weights arrive before term-2's x-lo
                nc.sync.dma_start(wq[0][1][:], wqh_lo[:, w0:w1])
                nc.sync.dma_start(wk[0][1][:], wkh_lo[:, w0:w1])
                nc.sync.dma_start(wq[1][1][:], wqh_lo[:, w1:w1 + D])
                nc.sync.dma_start(wk[1][1][:], wkh_lo[:, w1:w1 + D])
                for hf in range(2):
                    nc.sync.dma_start(
                        xl_t[:, hd * hf:hd * (hf + 1), :],
                        xTl[1024 * hf:1024 * (hf + 1), 0:TBE].rearrange(
                            "(j p) c -> p j c", p=128))
                xts[0] = (xh_t, xl_t)
                if p2 == 0:
                    # only tb0's cos/sin chunk up front; later chunks stream
                    # per-t-block so the x prefetches aren't queued behind
                    # 1MB of consts on the serial DMA device
                    nc.sync.dma_start(cos_sb[:, 0:TBE], cosT[:, 0:TBE])
                    nc.sync.dma_start(sin_sb[:, 0:TBE], sinTm[:, 0:TBE])
                nc.sync.dma_start(wv[0][:], wvh_hi[:, 2 * D * p2:2 * D * (p2 + 1)])
                nc.sync.dma_start(wv[1][:], wvh_lo[:, 2 * D * p2:2 * D * (p2 + 1)])
                if p2 == 0:
                    nc.sync.dma_start(rot_sb[:], rotm[:])
                    nc.sync.dma_start(bm_sb[:], bmask[:])
                    nc.sync.dma_start(id_sb[:], ident[:])
                else:
                    # prefetch the projection weights during pair-1 qkv
                    for hh in range(HPC):
                        nc.sync.dma_start(
                            wph_sb[:, hh, :], wpT_hi[128 * hh:128 * (hh + 1), :])
                        nc.sync.dma_start(
                            wpl_sb[:, hh, :], wpT_lo[128 * hh:128 * (hh + 1), :])

                for tb in range(NTB):
                    t0, t1_ = TBE * tb, TBE * (tb + 1)
                    for qk, tb_, nm_, m2_, r1_ in loop_ropes:
                        rope_fin(qk, tb_, nm_, m2_, r1_)
                    loop_ropes.clear()
                    if tb + 1 < NTB:
                        dma_x(tb + 1)
                        if p2 == 0:
                            nt0, nt1 = TBE * (tb + 1), TBE * (tb + 2)
                            nc.sync.dma_start(cos_sb[:, nt0:nt1], cosT[:, nt0:nt1])
                            nc.sync.dma_start(sin_sb[:, nt0:nt1], sinTm[:, nt0:nt1])
                    xh_t, xl_t = xts.pop(tb)
                    ps_q0 = ps_mm.tile([128, TBE], F32, tag="mm", name=f"psq0_{p2}_{tb}")
                    ps_k0 = ps_mm.tile([128, TBE], F32, tag="mm", name=f"psk0_{p2}_{tb}")
                    ps_q1 = ps_mm.tile([128, TBE], F32, tag="mm", name=f"psq1_{p2}_{tb}")
                    ps_k1 = ps_mm.tile([128, TBE], F32, tag="mm", name=f"psk1_{p2}_{tb}")
                    nvp = (TTPB + 1) // 2
                    ps_vs = [
                        ps_sm.tile([128, 512], F32, tag="sm", name=f"psv{p2}_{tb}_{i}")
                        for i in range(nvp)
                    ]
                    # 3-term fp8 accumulation: wh*xh + wl*xh + wh*xl,
                    # term-major (matches the tb-0 DMA arrival order); each
                    # psum parks right after its last matmul so the bank
                    # frees and the copy overlaps the remaining matmuls.
                    last_tb = tb == NTB - 1
                    qk_psums = ((ps_q0, wq[0], q_sbs[0]), (ps_k0, wk[0], k_sbs[0]),
                                (ps_q1, wq[1], q_sbs[1]), (ps_k1, wk[1], k_sbs[1]))

                    def qk_sweep():
                        for term in range(3):
                            wlv = 1 if term == 1 else 0
                            xt = xl_t if term == 2 else xh_t
                            first = term == 0
                            last = term == 2
                            for i_, (ps, wt, dst) in enumerate(qk_psums):
                                for j in range(NJ):
                                    js = slice(2 * j, 2 * j + 2)
                                    nc.tensor.matmul(ps[:], wt[wlv][:, js, :], xt[:, js, :],
                                                     start=(first and j == 0),
                                                     stop=(last and j == NJ - 1),
                                                     perf_mode=DR)
                                if last:
                                    # split parks across Act/DVE once
                                    # attention exps share the Act queue
                                    if tb > 0 and i_ % 2 == 1:
                                        nc.vector.tensor_copy(dst[:, t0:t1_], ps[:])
                                    else:
                                        nc.scalar.copy(dst[:, t0:t1_], ps[:])

                    def v_sweep():
                        for term in range(3):
                            wvt = wv[1 if term == 1 else 0]
                            xt = xl_t if term == 2 else xh_t
                            first = term == 0
                            last = term == 2
                            for j in range(NJ):
                                js = slice(2 * j, 2 * j + 2)
                                for tt in range(TTPB):
                                    nc.tensor.matmul(
                                        ps_vs[tt // 2][:, 256 * (tt % 2):256 * (tt % 2) + 256],
                                        xt[:, js, 128 * tt:128 * (tt + 1)],
                                        wvt[:, js, :],
                                        start=(first and j == 0 and tt % 2 == 0),
                                        stop=(last and j == NJ - 1),
                                        skip_group_check=True, perf_mode=DR)
                        for tt in range(TTPB):
                            gt = tb * TTPB + tt
                            o0 = 256 * (tt % 2)
                            vc = nc.vector.tensor_copy if (tb > 0 and tt % 2) else nc.scalar.copy
                            vc(vext[0][:, gt, 0:128], ps_vs[tt // 2][:, o0:o0 + 128])
                            vc(vext[1][:, gt, 0:128], ps_vs[tt // 2][:, o0 + 128:o0 + 256])

                    # last t-block: V first so its psums drain under the q/k
                    # matmuls and the pair boundary ends on parks+ropes only
                    if last_tb and tb > 0:
                        v_sweep()
                        qk_sweep()
                    else:
                        qk_sweep()
                        v_sweep()
                    # RoPE policy: in-loop ropes are queued and ISSUED at the
                    # start of the next t-block slot, so their rot matmuls
                    # land ahead of the 15us qkv stream in the in-order PE
                    # queue (a rope add stuck behind a whole t-block blocks
                    # every later DVE op).  Pair 0 rotates all four heads
                    # in-loop; pair 1 only its h2 head (h3's rotate in phase
                    # 3 via scheduler-placed consume_ropes).  The last
                    # t-block always defers.
                    if tb != NTB - 1:
                        heads = [0, 1] if p2 == 1 else [0]
                        for par in heads:
                            for qk, pfx in ((q_sbs[par], "q"), (k_sbs[par], "k")):
                                nm = f"{pfx}{par}_{p2}_{tb}"
                                m2_, r1_ = rope_mul(qk, tb, nm, eng=nc.gpsimd)
                                loop_ropes.append((qk, tb, nm, m2_, r1_))
                        if p2 == 0:
                            pend_ropes.append((q_sbs[1], tb, f"q1_{p2}_{tb}"))
                            pend_ropes.append((k_sbs[1], tb, f"k1_{p2}_{tb}"))
                    else:
                        # the pair's own (h-even) tail ropes go to the FRONT
                        # so schedule-placed consume_ropes() can rotate them
                        # before the partner-head ones
                        pend_ropes.insert(0, (k_sbs[0], tb, f"k0_{p2}_{tb}"))
                        pend_ropes.insert(0, (q_sbs[0], tb, f"q0_{p2}_{tb}"))
                        pend_ropes.append((q_sbs[1], tb, f"q1_{p2}_{tb}"))
                        pend_ropes.append((k_sbs[1], tb, f"k1_{p2}_{tb}"))
                    yield

            def attention_steps(h, quotas, pipelined, do_proj=False):
                """Generator for head h's attention, yielding once per chunk."""
                par = h % 2
                bufs = pair_bufs[h // 2]
                q_sb, k_sb = bufs["q"][par], bufs["k"][par]
                ve = bufs["ve"][par]
                if par == 0 and pend_ropes:
                    mine = [e for e in pend_ropes if e[0] is q_sb or e[0] is k_sb]
                    rest = [e for e in pend_ropes if not (e[0] is q_sb or e[0] is k_sb)]
                    pend_ropes[:] = mine + rest

                def stage_a(ib):
                    """scores + exp + diagonal mask for i-block ib; the score
                    matmul and exp narrow to the causal range on diagonal
                    j-tiles."""
                    i0 = sb * ib
                    jt_max = (i0 + sb) // 128 - 1  # inclusive
                    pts = [None] * (jt_max + 1)
                    for jt in range(jt_max + 1):
                        m = jt - NIC * ib
                        off = 128 * m if m > 0 else 0
                        s_ps = ps_mm.tile([128, sb], F32, tag="mm", name=f"s{h}_{ib}_{jt}")
                        nc.tensor.matmul(
                            s_ps[:, off:sb],
                            k_sb[:, 128 * jt:128 * (jt + 1)],
                            q_sb[:, i0 + off:i0 + sb],
                            start=True, stop=True)
                        pt_t = ptp.tile([128, sb], BF16, tag="pt", name=f"pt{h}_{ib}_{jt}")
                        nc.scalar.activation(pt_t[:, off:sb], s_ps[:, off:sb], AF.Exp, scale=SCALE)
                        if m >= 0:
                            pm = pt_t[:, 128 * m:128 * (m + 1)]
                            nc.vector.tensor_mul(pm, pm, bm_sb[:])
                        pts[jt] = pt_t
                    return pts

                def stage_b(ib, pts, bi):
                    """PV + normalize + transpose + fp8 hi/lo split."""
                    i0 = sb * ib

                    def finish(ic, pv):
                        rc = smallp.tile([128, 1], F32, tag="rc", name=f"rc{h}_{ib}_{ic}")
                        nc.vector.reciprocal(rc[:], pv[:, 128:129])
                        o_sb = smallp.tile([128, 128], BF16, tag="o", name=f"o{h}_{ib}_{ic}")
                        nc.vector.tensor_scalar_mul(o_sb[:], pv[:, 0:128], rc[:])
                        ot_ps = ps_mm.tile([128, 128], BF16, tag="mm", name=f"otp{h}_{ib}_{ic}")
                        nc.tensor.transpose(ot_ps[:], o_sb[:], id_sb[:])
                        c0 = i0 + 128 * ic
                        hs = oTh_sb[:, h, c0:c0 + 128]
                        if do_proj:
                            # proj-critical head: split straight off the psum
                            # on Act + DVE (no Pool latency in the chain)
                            nc.scalar.copy(hs, ot_ps[:])
                            nc.vector.scalar_tensor_tensor(
                                oTl_sb[:, h, c0:c0 + 128], hs, -1.0, ot_ps[:],
                                mybir.AluOpType.mult, mybir.AluOpType.add)
                        else:
                            otb = smallp.tile([128, 128], BF16, tag="otb", name=f"otb{h}_{ib}_{ic}")
                            nc.vector.tensor_copy(otb[:], ot_ps[:])
                            # fp8 hi/lo split on GpSimd (SBUF-only engine)
                            nc.gpsimd.tensor_copy(hs, otb[:])
                            nc.gpsimd.tensor_sub(oTl_sb[:, h, c0:c0 + 128], otb[:], hs)

                    prev = None
                    for ic in range(NIC):
                        last_jt = NIC * ib + ic
                        pv = ps_sm.tile([128, 129], F32, tag="sm", name=f"pv{h}_{ib}_{ic}")
                        for jt in range(last_jt + 1):
                            nc.tensor.matmul(
                                pv[:],
                                pts[jt][:, 128 * ic:128 * (ic + 1)],
                                ve[:, jt, :],
                                start=(jt == 0), stop=(jt == last_jt))
                        if prev is not None:
                            finish(*prev)
                            if do_proj:
                                emit_proj(NIC * ib + prev[0], on_act=False,
                                          tail=(ib == NSB - 1))
                        prev = (ic, pv)
                    finish(*prev)
                    if do_proj:
                        emit_proj(NIC * ib + prev[0],
                                  on_act=(ib == NSB - 1), tail=(ib == NSB - 1))
                    consume_ropes(quotas[bi])

                if pipelined:
                    pts_prev = None
                    for ib in range(NSB):
                        pts_cur = stage_a(ib)
                        yield
                        if pts_prev is not None:
                            stage_b(ib - 1, pts_prev, ib - 1)
                            yield
                        pts_prev = pts_cur
                    stage_b(NSB - 1, pts_prev, NSB - 1)
                    yield
                else:
                    for ib in range(NSB):
                        pts = stage_a(ib)
                        yield
                        stage_b(ib, pts, ib)
                        yield

            def stepn(g, n):
                for _ in range(n):
                    next(g)

            def run(g):
                for _ in g:
                    pass

            def consume_ropes(n):
                for qk, tb_, nm_ in pend_ropes[:n]:
                    rope_ip(qk, tb_, nm_)
                del pend_ropes[:n]

            # ---- schedule -------------------------------------------
            # pair 0 qkv alone; h0/h1 attention chunks (mutually interleaved
            # so one head's scores fill the other's exp latency) interleaved
            # with pair 1's qkv t-blocks; h2/h3 likewise interleaved with
            # each other and the output projection folded into h3's B chunks.
            if NTB >= 4 and NSB >= 4:
                Z = [0, 0, 0, 0]
                # h0's attention starts one i-block behind pair-0's qkv
                # t-blocks (its k/v prefix is complete by then), filling
                # pair-0's otherwise idle Act with exps; deferred ropes are
                # consumed at explicit schedule points, each before any chunk
                # that reads the rotated tile.
                q0 = qkv_pair_steps(0)
                a0 = attention_steps(0, Z, True)
                stepn(q0, 2)        # tb0 tb1 (tb0's ropes issue at tb1 start)
                stepn(a0, 1)        # A0
                stepn(q0, 1)        # tb2 (ropes tb1)
                stepn(a0, 2)        # A1 B0
                stepn(q0, 1)        # tb3 (ropes tb2)
                consume_ropes(2)    # q0/k0 pair0-tb3
                stepn(a0, 3)        # A2 B1 A3
                consume_ropes(4)    # q1/k1 pair0-tb0, tb1
                a1 = attention_steps(1, Z, True)
                q1 = qkv_pair_steps(1)
                a2 = attention_steps(2, Z, True)
                stepn(a0, 1)        # B2
                stepn(a1, 1)        # A0
                stepn(q1, 1)        # pair1 tb0
                consume_ropes(2)    # q1/k1 pair0-tb2
                stepn(a1, 2)        # A1 B0
                stepn(a0, 1)        # B3
                stepn(q1, 1)        # tb1 (ropes p1-tb0)
                consume_ropes(2)    # q1/k1 pair0-tb3
                stepn(a2, 1)        # h2.A0
                stepn(a1, 2)        # A2 B1
                stepn(q1, 1)        # tb2 (ropes p1-tb1)
                stepn(a2, 2)        # h2.A1 B0
                stepn(a1, 2)        # A3 B2
                stepn(q1, 1)        # tb3 (ropes p1-tb2)
                consume_ropes(2)    # q0/k0 pair1-tb3
                stepn(a2, 2)        # h2.A2 B1
                run(a1)             # B3
                run(a0)
                run(q0)
                run(q1)
                # h3's first chunks (and ib0's projection) pull into the
                # phase-2 tail where Act still has headroom; phase 3 is the
                # remainder with h2's tail interleaved.
                a3 = attention_steps(3, Z, True, do_proj=True)
                consume_ropes(2)    # q1/k1 pair1-tb3
                stepn(a3, 3)        # A0 A1 B0 (+proj ib0)
                stepn(a2, 1)        # h2.A3
                stepn(a2, 1)        # h2.B2
                stepn(a3, 2)        # A2 B1 (+proj ib1)
                stepn(a2, 1)        # h2.B3
                stepn(a3, 1)        # A3
                run(a3)             # B2 B3 (+proj ib2, ib3)
                run(a2)
            else:
                run(qkv_pair_steps(0))
                run(attention_steps(0, [2, 2, 0, 0], True))
                run(attention_steps(1, [0, 0, 0, 0], True))
                run(qkv_pair_steps(1))
                run(attention_steps(2, [2, 2, 0, 0], True))
                run(attention_steps(3, [0, 0, 0, 0], True, do_proj=True))

    nc.compile()
    return nc


def host_consts(t=T):
    """RoPE cos/sin (scaled by 1/WS to fold out the fp8 weight pre-scale),
    causal big-mask, identity, signed half-rotation."""
    inv = (1.0 / (np.float32(10000.0) ** (np.arange(0, DH, 2, dtype=np.float32) / np.float32(DH)))).astype(np.float32)
    tt = np.arange(t, dtype=np.float32)
    fr = np.outer(tt, inv).astype(np.float32)       # [t, 64]
    emb = np.concatenate([fr, fr], axis=1)          # [t, 128]
    cosT = np.ascontiguousarray(np.cos(emb).T.astype(np.float32)) / np.float32(WS)
    sinTm = np.ascontiguousarray(np.sin(emb).T.astype(np.float32)) / np.float32(WS)
    jj = np.arange(128)[:, None]
    cc = np.arange(128)[None, :]
    bmask = (cc >= jj).astype(np.float32)
    ident = np.eye(128, dtype=np.float32)
    # signed half-rotation: (rotm.T @ x)[d] = -x[d+64] for d<64, x[d-64] else
    rotm = np.zeros((128, 128), dtype=np.float32)
    for d in range(64):
        rotm[d + 64, d] = -1.0
        rotm[d, d + 64] = 1.0
    return cosT, sinTm, bmask, ident, rotm


def _warrange(w):
    """[128*nh rows, D] head-major weight slice -> [128, nh*D] sbuf-ready layout:
    block h, col di*128+c of partition p  =  w[128*h + c, 128*di + p]."""
    nh = w.shape[0] // 128
    d = w.shape[1]
    out = np.empty((128, nh * d), dtype=w.dtype)
    for h in range(nh):
        a = w[128 * h:128 * (h + 1), :].T.reshape(d // 128, 128, 128)  # [di, p, c]
        out[:, d * h:d * (h + 1)] = a.transpose(1, 0, 2).reshape(128, d)
    return out


def _wvarrange(w):
    """[512 rows, D] 4-head v-weights -> [128, 2*2*D]: per pair, di-major blocks of
    [even-head 128 cols | odd-head 128 cols]."""
    d = w.shape[1]
    blocks = []
    for p2 in range(2):
        e = w[256 * p2:256 * p2 + 128, :].T.reshape(d // 128, 128, 128)
        o = w[256 * p2 + 128:256 * p2 + 256, :].T.reshape(d // 128, 128, 128)
        pair = np.concatenate([e, o], axis=2)          # [di, p, 256]
        blocks.append(pair.transpose(1, 0, 2).reshape(128, 2 * d))
    return np.concatenate(blocks, axis=1)


FP8NP = ml_dtypes.float8_e4m3


def _split8(a):
    """Error-compensated e4m3 split: a ~= hi + lo (fp32 in, fp8 out pair)."""
    a = np.asarray(a, dtype=np.float32)
    hi = a.astype(FP8NP)
    lo = (a - hi.astype(np.float32)).astype(FP8NP)
    return hi, lo


def shard_inputs(x, w_qkv, w_proj, t=T, pv_dt="bfloat16"):
    """Build the 8 per-core input maps (host does the fp8 splits)."""
    bdt = ml_dtypes.bfloat16
    cosT, sinTm, bmask, ident, rotm = host_consts(t)
    cosT = cosT.astype(bdt)
    sinTm = sinTm.astype(bdt)
    bmask = bmask.astype(bdt)
    ident = ident.astype(bdt)
    rotm = rotm.astype(bdt)
    d = x.shape[2]
    ws = np.float32(WS)
    in_maps = []
    xs = {}
    for b in range(x.shape[0]):
        xs[b] = _split8(np.ascontiguousarray(x[b].T))
    for c in range(8):
        b, g = divmod(c, 4)
        s0, s1 = 512 * g, 512 * (g + 1)
        wq_hi, wq_lo = _split8(w_qkv[s0:s1, :] * ws)
        wk_hi, wk_lo = _split8(w_qkv[d + s0:d + s1, :] * ws)
        wv_hi, wv_lo = _split8(w_qkv[2 * d + s0:2 * d + s1, :] * ws)
        wp_hi, wp_lo = _split8(np.ascontiguousarray(w_proj[:, s0:s1].T) * ws)
        in_maps.append(dict(
            xTh=xs[b][0], xTl=xs[b][1],
            wqh_hi=_warrange(wq_hi), wqh_lo=_warrange(wq_lo),
            wkh_hi=_warrange(wk_hi), wkh_lo=_warrange(wk_lo),
            wvh_hi=_wvarrange(wv_hi), wvh_lo=_wvarrange(wv_lo),
            wpT_hi=wp_hi, wpT_lo=wp_lo,
            cosT=cosT, sinTm=sinTm, bmask=bmask, ident=ident, rotm=rotm,
        ))
    return in_maps


_NC_CACHE = {}


def get_nc(t=T, mm_dt="float32r", pv_dt="bfloat16"):
    key = (t, mm_dt, pv_dt)
    if key not in _NC_CACHE:
        _NC_CACHE[key] = build_nc(t=t, mm_dt=mm_dt, pv_dt=pv_dt)
    return _NC_CACHE[key]


def kernel(x, w_qkv, w_proj):
    x = np.asarray(x, dtype=np.float32)
    w_qkv = np.asarray(w_qkv, dtype=np.float32)
    w_proj = np.asarray(w_proj, dtype=np.float32)
    b_, t_, d_ = x.shape
    in_maps = shard_inputs(x, w_qkv, w_proj, t=t_)
    nc = get_nc(t=t_)
    res = run_bass_kernel_spmd(nc, in_maps, list(range(8))).results
    out = np.zeros((b_, t_, d_), dtype=np.float32)
    for c in range(8):
        b, _ = divmod(c, 4)
        out[b] += res[c]["y"]
    return out
